# revision 1
# baseline (speedup 1.0000x reference)
"""GCN 3-layer forward on 8 Trainium2 NeuronCores (Bass/Tile).

Self-contained: hardcodes the problem shapes from the spec.
kernel(**inputs) -> np.ndarray [50000, 128] float32.

Layout: feature-major ("transposed") on chip — features on partitions,
nodes along the free dim.  Nodes are degree-sorted and round-robin
assigned to cores; per-core slots sorted by A-half degree so the
segmented reduce is a short list of constant-degree runs shared by all
cores (histograms padded to a common shape).  Message gather uses
dma_gather(transpose=True) from f16 tables in DRAM; the int16 index
limit is handled by an A/B split of the table (cores 0-3 via view
[0,32768), cores 4-7 via [TR-32768,TR)).  The B-phase output is
permuted back to slot order with ap_gather over fp32-paired f16
columns.  Symmetric normalization is factorized: table rows pre-scaled
by dinv[src], aggregates post-scaled by dinv[dst].  Conv bias is
dropped (cancels exactly in training-mode BatchNorm).  BN+LeakyReLU is
one ScalarE activation; BN stats ride accum_out + a tiny AllReduce;
tables are shared with AllGather.
"""
import sys

sys.path.insert(0, "/opt/trn_rl_repo")

import numpy as np
import ml_dtypes

import concourse.bacc as bacc
import concourse.mybir as mybir
import concourse.tile as tile
from concourse.bass_utils import run_bass_kernel_spmd

F16 = np.float16

N, E, DIN, DH, DOUT = 50000, 800000, 128, 256, 128
EPS = 1e-5
SLOPE = 0.01
NCORES = 8
ACORES = 4          # cores 0..3 form the "A" half of the table
CH = 896            # idxs per 512B-elem gather call (SWDGE ring limit)
SUB = 4             # 512B gather sub-calls per reduce chunk
CHB = CH * SUB      # edges per reduce chunk
CHA = 896           # idxs per 256B-elem gather call
SUBA = 4            # 256B gather sub-calls per reduce chunk
IMAX = 32768        # int16 index window


# ----------------------------------------------------------------------------
# host-side schedule construction
# ----------------------------------------------------------------------------

def _wrap_idx(arr):
    """1-D int array -> [128, len/16] int16 wrapped+replicated layout."""
    L = len(arr)
    assert L % 16 == 0
    a = np.asarray(arr, np.int16).reshape(L // 16, 16).T  # [16, L/16]
    return np.tile(a, (8, 1)).copy()  # [128, L/16]


def _chunk_cols(gvec, cap):
    """Split columns into chunks of <= cap edges, breaking at even column
    indices only (keeps 4B alignment for the 2x DVE reduce mode)."""
    chunks = []
    lo = 0
    acc = 0
    j = 0
    n = len(gvec)
    while j < n:
        g2 = gvec[j] + (gvec[j + 1] if j + 1 < n else 0)
        if acc + g2 > cap and acc > 0:
            chunks.append((lo, j))
            lo = j
            acc = 0
        acc += g2
        j += 2
    chunks.append((lo, n))
    return chunks


def _runs_for_chunk(gvec, lo, hi):
    """Consecutive constant-degree runs (skipping g==0) within cols [lo,hi).
    Returns (in_off, out_col, n, g); in_off relative to chunk start."""
    runs = []
    off = 0
    j = lo
    while j < hi:
        g = gvec[j]
        k = j
        while k < hi and gvec[k] == g:
            k += 1
        if g > 0:
            runs.append((int(off), int(j), int(k - j), int(g)))
        off += g * (k - j)
        j = k
    return runs


def _phase_schedule(gvec, cap):
    chunks = _chunk_cols(gvec, cap)
    out = []
    for (lo, hi) in chunks:
        out.append({"cols": (lo, hi),
                    "nedges": int(np.sum(gvec[lo:hi])),
                    "runs": _runs_for_chunk(gvec, lo, hi)})
    return out


def _col_positions(sched, gvec, ch):
    pos = np.zeros(len(gvec), np.int64)
    for k, chk in enumerate(sched):
        lo, hi = chk["cols"]
        off = 0
        for j in range(lo, hi):
            pos[j] = k * ch + off
            off += gvec[j]
    return pos


def preprocess(edge_index, x):
    src0 = np.asarray(edge_index[0], np.int64)
    dst0 = np.asarray(edge_index[1], np.int64)
    loop = np.arange(N, dtype=np.int64)
    src = np.concatenate([src0, loop])
    dst = np.concatenate([dst0, loop])

    deg = np.bincount(dst, minlength=N)
    dinv = (1.0 / np.sqrt(deg.astype(np.float64))).astype(np.float32)

    order = np.argsort(deg, kind="stable")
    core_of = np.empty(N, np.int32)
    core_of[order] = np.arange(N) % NCORES

    a_mask = core_of[src] < ACORES
    gA = np.bincount(dst[a_mask], minlength=N)
    gB = deg - gA
    gAp = gA + (gA & 1)   # even-padded phase degrees
    gBp = gB + (gB & 1)

    # per-core slot order: sorted by even A-degree
    gmaxA = int(gAp.max())
    counts_a = np.zeros((NCORES, gmaxA + 1), np.int64)
    percore_nodes = []
    for c in range(NCORES):
        nodes_c = np.flatnonzero(core_of == c)
        nodes_c = nodes_c[np.argsort(gAp[nodes_c], kind="stable")]
        percore_nodes.append(nodes_c)
        counts_a[c] = np.bincount(gAp[nodes_c], minlength=gmaxA + 1)

    mA = counts_a.max(axis=0)
    mA[0] += 2                        # guarantee pad slots (zero table rows)
    mA += mA & 1                      # even bucket sizes
    nc_raw = int(mA.sum())
    NC = -(-nc_raw // 512) * 512
    mA[0] += NC - nc_raw

    gvecA = np.repeat(np.arange(gmaxA + 1), mA)
    startA = np.concatenate([[0], np.cumsum(mA)])

    slot_node = []
    for c in range(NCORES):
        sn = np.full(NC, -1, np.int64)
        nodes_c = percore_nodes[c]
        gs = gAp[nodes_c]
        ranks = np.arange(len(nodes_c)) - np.searchsorted(gs, gs)
        sn[startA[gs] + ranks] = nodes_c
        slot_node.append(sn)

    slot_of = np.full(N, -1, np.int64)
    for c in range(NCORES):
        real = slot_node[c] >= 0
        slot_of[slot_node[c][real]] = np.flatnonzero(real)

    TR = 2 + NCORES * NC
    baseB = max(0, TR - IMAX)
    row_of = 1 + core_of.astype(np.int64) * NC + slot_of

    assert int(row_of[core_of < ACORES].max()) < IMAX
    assert int(row_of[core_of >= ACORES].min()) >= baseB

    # --- B phase: slots paired (2p, 2p+1); both columns padded to the
    # pair's max even B-degree so ap_gather can move fp32-paired columns.
    gB_slot = np.zeros((NCORES, NC), np.int64)
    for c in range(NCORES):
        real = slot_node[c] >= 0
        gB_slot[c][real] = gBp[slot_node[c][real]]
    pairdeg = gB_slot.reshape(NCORES, NC // 2, 2).max(axis=2)  # [cores, NP]
    NP = NC // 2
    gmaxB = int(pairdeg.max())
    counts_b = np.zeros((NCORES, gmaxB + 1), np.int64)
    for c in range(NCORES):
        counts_b[c] = np.bincount(pairdeg[c], minlength=gmaxB + 1)
    mB = counts_b.max(axis=0)
    npb_raw = int(mB.sum())
    NPB = -(-npb_raw // 16) * 16
    mB[0] += NPB - npb_raw
    NCB = 2 * NPB
    pairdegvec = np.repeat(np.arange(gmaxB + 1), mB)   # per B-pair degree
    gvecB = np.repeat(pairdegvec, 2)                   # per B-column degree
    startBp = np.concatenate([[0], np.cumsum(mB)])

    paircol = np.zeros((NCORES, NP), np.int64)  # slot-pair -> B-pair pos
    for c in range(NCORES):
        pd = pairdeg[c]
        o = np.argsort(pd, kind="stable")
        ranks = np.arange(NP) - np.searchsorted(pd[o], pd[o])
        pc = np.empty(NP, np.int64)
        pc[o] = startBp[pd[o]] + ranks
        paircol[c] = pc

    zA = int(startA[1])                   # row of (core 0, slot m0A-1)
    zB = int(7 * NC + startA[1])          # row of (core 7, slot m0A-1)
    schedA = _phase_schedule(gvecA, CHB)
    schedB = _phase_schedule(gvecB, CHB)
    EA, EB = len(schedA) * CHB, len(schedB) * CHB

    colposA = _col_positions(schedA, gvecA, CHB)
    colposB = _col_positions(schedB, gvecB, CHB)

    idxA, idxB, apgb_l, dinvb_l = [], [], [], []
    for c in range(NCORES):
        # --- A stream (default idx 0 = zero row)
        sA = np.full(EA, zA, np.int64)
        m = (core_of[dst] == c) & a_mask
        es, cols = src[m], slot_of[dst[m]]
        o = np.argsort(cols, kind="stable")
        es, cols = es[o], cols[o]
        ranks = np.arange(len(cols)) - np.searchsorted(cols, cols)
        sA[colposA[cols] + ranks] = row_of[es]
        assert sA.max() < IMAX
        idxA.append(_wrap_idx(sA))

        # --- B stream (default = B zero row)
        sB = np.full(EB, zB - baseB, np.int64)
        m = (core_of[dst] == c) & (~a_mask)
        es, sl = src[m], slot_of[dst[m]]
        cols = paircol[c][sl // 2] * 2 + (sl & 1)
        o = np.argsort(cols, kind="stable")
        es, cols = es[o], cols[o]
        ranks = np.arange(len(cols)) - np.searchsorted(cols, cols)
        sB[colposB[cols] + ranks] = row_of[es] - baseB
        assert sB.min() >= 0 and sB.max() < IMAX
        idxB.append(_wrap_idx(sB))

        apgb_l.append(_wrap_idx(paircol[c]))

        dv = np.zeros(NC, np.float32)
        real = slot_node[c] >= 0
        dv[real] = dinv[slot_node[c][real]]
        dinvb_l.append(np.tile(dv[None, :].astype(F16), (128, 1)))

    xt = np.zeros((TR, DIN), F16)
    real_nodes = slot_of >= 0
    xs = (np.asarray(x, np.float32) * dinv[:, None]).astype(F16)
    xt[row_of] = xs

    sched = {
        "NC": NC, "NCB": NCB, "NPB": NPB, "TR": TR, "baseB": baseB,
        "schedA": schedA, "schedB": schedB, "EA": EA, "EB": EB,
        "m0A": int(startA[1]), "m0B": int(2 * startBp[1]),
        "slot_node": slot_node,
    }
    data = {"xt": xt, "idxA": idxA, "idxB": idxB, "apgb": apgb_l,
            "dinvb": dinvb_l}
    return sched, data


# ----------------------------------------------------------------------------
# device kernel builder
# ----------------------------------------------------------------------------

def build_nc(sched, debug=False):
    NC, NCB, TR = sched["NC"], sched["NCB"], sched["TR"]
    NPB = sched["NPB"]
    baseB = sched["baseB"]
    EA, EB = sched["EA"], sched["EB"]
    fp32 = mybir.dt.float32
    f16 = mybir.dt.float16
    i16 = mybir.dt.int16
    AF = mybir.ActivationFunctionType
    OP = mybir.AluOpType
    NCH = NC // 512

    nc = bacc.Bacc("TRN2", target_bir_lowering=False, num_devices=NCORES,
                   num_swdge_queues=4, dynamic_dma_scratch_size=30720)

    xt_d = nc.dram_tensor("xt", [TR, DIN], f16, kind="ExternalInput")
    idxa_d = nc.dram_tensor("idxa", [128, EA // 16], i16, kind="ExternalInput")
    idxb_d = nc.dram_tensor("idxb", [128, EB // 16], i16, kind="ExternalInput")
    apgb_d = nc.dram_tensor("apgb", [128, NC // 32], i16, kind="ExternalInput")
    dinvb_d = nc.dram_tensor("dinvb", [128, NC], f16, kind="ExternalInput")
    w0_d = nc.dram_tensor("w0", [128, 256], f16, kind="ExternalInput")
    w1_d = nc.dram_tensor("w1", [128, 512], f16, kind="ExternalInput")
    w2_d = nc.dram_tensor("w2", [128, 256], f16, kind="ExternalInput")
    bnp_d = nc.dram_tensor("bnp", [128, 10], fp32, kind="ExternalInput")
    identb_d = nc.dram_tensor("identb", [128, 128], f16, kind="ExternalInput")
    identf_d = nc.dram_tensor("identf", [128, 128], fp32, kind="ExternalInput")
    out_d = nc.dram_tensor("out", [NC, DOUT], fp32, kind="ExternalOutput")
    dbg = {}
    if debug:
        for name, shape in [
            ("dbg_agg0", [128, NC]), ("dbg_cv0", [128, 2 * NC]),
            ("dbg_h1", [128, 2 * NC]), ("dbg_agg1", [128, 2 * NC]),
            ("dbg_st0", [128, 4]),
        ]:
            dbg[name] = nc.dram_tensor(name, shape, fp32,
                                       kind="ExternalOutput")

    with tile.TileContext(nc) as tc:
        with (
            tc.tile_pool(name="const", bufs=1) as constp,
            tc.tile_pool(name="gat", bufs=2) as gatp,
            tc.tile_pool(name="red", bufs=1) as redp,
            tc.tile_pool(name="agg", bufs=2) as aggp,
            tc.tile_pool(name="perm", bufs=1) as permp,
            tc.tile_pool(name="small", bufs=2) as smallp,
            tc.tile_pool(name="rowt", bufs=2) as rowp,
            tc.tile_pool(name="ps", bufs=3, space="PSUM") as psp,
            tc.tile_pool(name="pst", bufs=2, space="PSUM") as pstp,
            tc.tile_pool(name="pstf", bufs=2, space="PSUM") as pstfp,
            tc.tile_pool(name="psq", bufs=1, space="PSUM") as psqp,
            tc.tile_pool(name="dram", bufs=1, space="DRAM") as dramp,
        ):
            # ---- resident constants
            apgb = constp.tile([128, NC // 32], i16, tag="apgb")
            dinvb = constp.tile([128, NC], f16, tag="dinvb")
            w0 = constp.tile([128, 256], f16, tag="w0")
            w1 = constp.tile([128, 512], f16, tag="w1")
            w2 = constp.tile([128, 256], f16, tag="w2")
            bnp = constp.tile([128, 10], fp32, tag="bnp")
            identb = constp.tile([128, 128], f16, tag="identb")
            identf = constp.tile([128, 128], fp32, tag="identf")
            for t, d in [(apgb, apgb_d),
                         (dinvb, dinvb_d), (w0, w0_d), (w1, w1_d),
                         (w2, w2_d), (bnp, bnp_d), (identb, identb_d),
                         (identf, identf_d)]:
                nc.sync.dma_start(t[:], d[:])

            tbl = dramp.tile([TR, DH], f16, tag="tbl", addr_space="Shared")
            tbl2 = dramp.tile([TR, DOUT], f16, tag="tbl2",
                              addr_space="Shared")
            agsrc = dramp.tile([NC, DH], f16, tag="agsrc")
            agsrc2 = dramp.tile([NC, DOUT], f16, tag="agsrc2")

            def gather_reduce(table, elem, blocks):
                """A+B gather phases -> (outA f16, outB f16) [128,2,*]."""
                outA = redp.tile([128, 2, NC], f16, tag="outA")
                outB = redp.tile([128, 2, NCB], f16, tag="outB")
                with nc.allow_low_precision(reason="DVE accumulates fp32"):
                    for phase in ("A", "B"):
                        if phase == "A":
                            idxd, sch, outX, m0 = \
                                idxa_d, sched["schedA"], outA, sched["m0A"]
                            view = table[0:min(TR, IMAX), :]
                        else:
                            idxd, sch, outX, m0 = \
                                idxb_d, sched["schedB"], outB, sched["m0B"]
                            view = table[baseB:TR, :]
                        for j in range(blocks):
                            if m0 > 0:
                                nc.vector.memset(outX[:, j, :m0], 0)
                        for k, chk in enumerate(sch):
                            gbs = [gatp.tile([128, 1, CHB], f16, tag=f"gb{j}",
                                             name=f"gb{j}")
                                   for j in range(blocks)]
                            idxt = gatp.tile([128, CHB // 16], i16,
                                             tag="idxt")
                            nc.sync.dma_start(
                                idxt[:], idxd[:, k * (CHB // 16):
                                              (k + 1) * (CHB // 16)])
                            for j in range(blocks):
                                vj = (view if blocks == 1
                                      else view[:, j * 128:(j + 1) * 128])
                                step = None if blocks == 1 else elem
                                for sx in range(SUBA):
                                    nc.gpsimd.dma_gather(
                                        out_ap=gbs[j][:, :,
                                                      sx * CHA:
                                                      (sx + 1) * CHA],
                                        in_ap=vj,
                                        idxs_ap=idxt[:, sx * (CHA // 16):
                                                     (sx + 1) * (CHA // 16)],
                                        num_idxs=CHA,
                                        num_idxs_reg=CHA,
                                        elem_size=128,
                                        elem_step=step,
                                        transpose=True,
                                        queue_num=(k * SUBA * blocks +
                                                   j * SUBA + sx) % 4,
                                    )
                            for (ioff, ocol, n, g) in chk["runs"]:
                                for j in range(blocks):
                                    nc.vector.tensor_reduce(
                                        outX[:, j, ocol:ocol + n],
                                        gbs[j][:, 0, ioff:ioff + n * g]
                                        .rearrange("p (n g) -> p n g", g=g),
                                        axis=mybir.AxisListType.X,
                                        op=OP.add,
                                    )
                return outA, outB

            def merge(outA, outB, blocks):
                """B-perm + add + dinv[dst] scale -> aggT f16 [128,2,NC]."""
                aggT = aggp.tile([128, 2, NC], f16, tag="aggbuf")
                for j in range(blocks):
                    tmp = permp.tile([128, NC // 2], fp32, tag="ptmp")
                    nc.gpsimd.ap_gather(
                        out_ap=tmp[:],
                        in_ap=outB[:, j, :].bitcast(fp32),
                        idxs_ap=apgb[:],
                        channels=128,
                        num_elems=NPB,
                        d=1,
                        num_idxs=NC // 2,
                    )
                    tb = tmp[:].bitcast(f16)
                    nc.vector.tensor_tensor(aggT[:, j, :], tb,
                                            outA[:, j, :], OP.add)
                    nc.vector.tensor_tensor(aggT[:, j, :], aggT[:, j, :],
                                            dinvb[:], OP.mult)
                return aggT

            def bn_consts(st, blocks_out, bn_off, layer):
                """AllReduce stats -> per-feature scale A / bias B tiles."""
                stin = dramp.tile([128, 4], fp32, tag=f"stin{layer}")
                stout = dramp.tile([128, 4], fp32, tag=f"stout{layer}",
                                   addr_space="Shared")
                nc.gpsimd.dma_start(stin[:], st[:])
                nc.gpsimd.collective_compute(
                    "AllReduce", OP.add,
                    replica_groups=[list(range(NCORES))],
                    ins=[stin.opt()], outs=[stout.opt()],
                )
                stg = smallp.tile([128, 4], fp32, tag="stg")
                nc.sync.dma_start(stg[:], stout[:])
                b = blocks_out
                mu = smallp.tile([128, 2], fp32, tag="mu")
                va = smallp.tile([128, 2], fp32, tag="va")
                Ab = smallp.tile([128, 2], fp32, tag="Ab")
                Bb = smallp.tile([128, 2], fp32, tag="Bb")
                musq = smallp.tile([128, 2], fp32, tag="musq")
                rstd = smallp.tile([128, 2], fp32, tag="rstd")
                nc.vector.tensor_scalar(mu[:, :b], stg[:, 0:b], 1.0 / N, None,
                                        op0=OP.mult)
                nc.vector.tensor_scalar(va[:, :b], stg[:, 2:2 + b], 1.0 / N,
                                        None, op0=OP.mult)
                nc.vector.tensor_tensor(musq[:, :b], mu[:, :b], mu[:, :b],
                                        OP.mult)
                nc.vector.tensor_tensor(va[:, :b], va[:, :b], musq[:, :b],
                                        OP.subtract)
                sqv = smallp.tile([128, 2], fp32, tag="sqv")
                nc.vector.tensor_scalar(sqv[:, :b], va[:, :b], EPS, None,
                                        op0=OP.add)
                nc.scalar.activation(sqv[:, :b], sqv[:, :b], AF.Sqrt)
                nc.vector.reciprocal(rstd[:, :b], sqv[:, :b])
                gsl = bnp[:, bn_off:bn_off + b]
                bsl = bnp[:, bn_off + b:bn_off + 2 * b]
                nc.vector.tensor_tensor(Ab[:, :b], rstd[:, :b], gsl, OP.mult)
                nc.vector.tensor_tensor(Bb[:, :b], mu[:, :b], Ab[:, :b],
                                        OP.mult)
                nc.vector.tensor_tensor(Bb[:, :b], bsl, Bb[:, :b],
                                        OP.subtract)
                return Ab, Bb

            def conv_bn(aggT, wt, KT, bn_off, lrelu, layer):
                """matmul (out 2 blocks of 128) + BN(+lrelu) -> h f16."""
                cv = aggp.tile([128, 2, NC], f16, tag="aggbuf")
                ssum = smallp.tile([128, 2, NCH], fp32, tag="ssum")
                sqsum = smallp.tile([128, 2, NCH], fp32, tag="sqsum")
                for j in range(2):
                    for t in range(NCH):
                        ps = psp.tile([128, 512], fp32, tag="cps")
                        sl = slice(t * 512, (t + 1) * 512)
                        for kt in range(KT):
                            lhsT = wt[:, kt * 256 + j * 128:
                                      kt * 256 + (j + 1) * 128]
                            nc.tensor.matmul(ps[:], lhsT, aggT[:, kt, sl],
                                             start=(kt == 0),
                                             stop=(kt == KT - 1))
                        sq = smallp.tile([128, 512], f16, tag="sqd")
                        nc.scalar.activation(cv[:, j, sl], ps[:], AF.Copy,
                                             accum_out=ssum[:, j, t:t + 1])
                        nc.scalar.activation(sq[:], ps[:], AF.Square,
                                             accum_out=sqsum[:, j, t:t + 1])
                st = smallp.tile([128, 4], fp32, tag="stl")
                for j in range(2):
                    nc.vector.tensor_reduce(st[:, j:j + 1], ssum[:, j, :],
                                            axis=mybir.AxisListType.X,
                                            op=OP.add)
                    nc.vector.tensor_reduce(st[:, 2 + j:3 + j], sqsum[:, j, :],
                                            axis=mybir.AxisListType.X,
                                            op=OP.add)
                Ab, Bb = bn_consts(st, 2, bn_off, layer)
                h = aggp.tile([128, 2, NC], f16, tag="aggbuf")
                fn = AF.Lrelu if lrelu else AF.Identity
                for j in range(2):
                    nc.scalar.activation(h[:, j, :], cv[:, j, :], fn,
                                         bias=Bb[:, j:j + 1],
                                         scale=Ab[:, j:j + 1], alpha=SLOPE)
                return h, cv, st

            def write_rows(srcT, blocks, dst_dram, width, prescale):
                """(optional dinv[src] prescale) + transpose + DMA rows."""
                if prescale:
                    hs = aggp.tile([128, 2, NC], f16, tag="aggbuf")
                    for j in range(blocks):
                        nc.vector.tensor_tensor(hs[:, j, :], srcT[:, j, :],
                                                dinvb[:], OP.mult)
                    srcT = hs
                for t in range(NC // 128):
                    row = rowp.tile([128, width], f16, tag="rowt")
                    for j in range(blocks):
                        pt = pstp.tile([128, 128], f16, tag="tps")
                        nc.tensor.transpose(
                            pt[:], srcT[:, j, t * 128:(t + 1) * 128],
                            identb[:])
                        nc.scalar.activation(row[:, j * 128:(j + 1) * 128],
                                             pt[:], AF.Copy)
                    nc.sync.dma_start(dst_dram[t * 128:(t + 1) * 128, :],
                                      row[:])
                return srcT

            # ================= layer 0 =================
            outA, outB = gather_reduce(xt_d, DIN, 1)
            aggT = merge(outA, outB, 1)
            if debug:
                nc.gpsimd.dma_start(dbg["dbg_agg0"][:], aggT[:, 0, :])
            h1, cv0, st0 = conv_bn(aggT, w0, 1, 0, True, 0)
            if debug:
                nc.sync.dma_start(dbg["dbg_st0"][:], st0[:])
                nc.gpsimd.dma_start(
                    dbg["dbg_cv0"][:].rearrange("p (a b) -> p a b", a=2),
                    cv0[:])
                nc.gpsimd.dma_start(
                    dbg["dbg_h1"][:].rearrange("p (a b) -> p a b", a=2),
                    h1[:])
            write_rows(h1, 2, agsrc, DH, prescale=True)
            nc.gpsimd.collective_compute(
                "AllGather", OP.bypass,
                replica_groups=[list(range(NCORES))],
                ins=[agsrc.opt()], outs=[tbl[1:1 + NCORES * NC, :]],
            )

            # ================= layer 1 =================
            outA, outB = gather_reduce(tbl, DH, 2)
            aggT = merge(outA, outB, 2)
            if debug:
                nc.gpsimd.dma_start(
                    dbg["dbg_agg1"][:].rearrange("p (a b) -> p a b", a=2),
                    aggT[:])
            h2, _, _ = conv_bn(aggT, w1, 2, 4, True, 1)
            # transform-first for layer 2: T2 = W2 @ (dinv * h2)
            hs2 = aggp.tile([128, 2, NC], f16, tag="aggbuf")
            for j in range(2):
                nc.vector.tensor_tensor(hs2[:, j, :], h2[:, j, :], dinvb[:],
                                        OP.mult)
            t2 = aggp.tile([128, 2, NC], f16, tag="aggbuf")
            for t in range(NCH):
                ps = psp.tile([128, 512], fp32, tag="cps")
                sl = slice(t * 512, (t + 1) * 512)
                for kt in range(2):
                    nc.tensor.matmul(ps[:], w2[:, kt * 128:(kt + 1) * 128],
                                     hs2[:, kt, sl],
                                     start=(kt == 0), stop=(kt == 1))
                nc.scalar.activation(t2[:, 0, sl], ps[:], AF.Copy)
            write_rows(t2, 1, agsrc2, DOUT, prescale=False)
            nc.gpsimd.collective_compute(
                "AllGather", OP.bypass,
                replica_groups=[list(range(NCORES))],
                ins=[agsrc2.opt()], outs=[tbl2[1:1 + NCORES * NC, :]],
            )

            # ================= layer 2 =================
            outA, outB = gather_reduce(tbl2, DOUT, 1)
            aggT = merge(outA, outB, 1)
            # aggT IS the conv output (transform-first); BN only, no lrelu.
            ssum = smallp.tile([128, 2, NCH], fp32, tag="ssum")
            sqsum = smallp.tile([128, 2, NCH], fp32, tag="sqsum")
            for t in range(NCH):
                sl = slice(t * 512, (t + 1) * 512)
                sq = smallp.tile([128, 512], f16, tag="sqd")
                nc.scalar.activation(sq[:], aggT[:, 0, sl], AF.Square,
                                     accum_out=sqsum[:, 0, t:t + 1])
                nc.vector.tensor_reduce(ssum[:, 0, t:t + 1], aggT[:, 0, sl],
                                        axis=mybir.AxisListType.X, op=OP.add)
            st = smallp.tile([128, 4], fp32, tag="stl")
            nc.vector.tensor_reduce(st[:, 0:1], ssum[:, 0, :],
                                    axis=mybir.AxisListType.X, op=OP.add)
            nc.vector.tensor_reduce(st[:, 2:3], sqsum[:, 0, :],
                                    axis=mybir.AxisListType.X, op=OP.add)
            nc.vector.memset(st[:, 1:2], 0)
            nc.vector.memset(st[:, 3:4], 0)
            Ab, Bb = bn_consts(st, 1, 8, 2)
            for t in range(NC // 128):
                hf = smallp.tile([128, 128], fp32, tag="hfin")
                nc.scalar.activation(hf[:], aggT[:, 0, t * 128:(t + 1) * 128],
                                     AF.Identity,
                                     bias=Bb[:, 0:1], scale=Ab[:, 0:1])
                row = rowp.tile([128, DOUT], fp32, tag="rowtf")
                pt = pstfp.tile([128, 128], fp32, tag="tpsf")
                nc.tensor.transpose(pt[:], hf[:], identf[:])
                nc.vector.tensor_copy(row[:], pt[:])
                nc.sync.dma_start(out_d[t * 128:(t + 1) * 128, :], row[:])

    nc.compile()
    return nc


# ----------------------------------------------------------------------------
# entry point
# ----------------------------------------------------------------------------

def _make_inmaps(sched, data, W0, W1, W2, g0, be0, g1, be1, g2, be2):
    w0 = np.ascontiguousarray(W0.T.astype(F16))
    w1 = np.ascontiguousarray(
        W1.T.reshape(2, 128, 256).transpose(1, 0, 2).reshape(128, 512)
        .astype(F16))
    w2 = np.ascontiguousarray(
        W2.T.reshape(2, 128, 128).transpose(1, 0, 2).reshape(128, 256)
        .astype(F16))
    bnp = np.zeros((128, 10), np.float32)
    bnp[:, 0:2] = g0.reshape(2, 128).T
    bnp[:, 2:4] = be0.reshape(2, 128).T
    bnp[:, 4:6] = g1.reshape(2, 128).T
    bnp[:, 6:8] = be1.reshape(2, 128).T
    bnp[:, 8] = g2
    bnp[:, 9] = be2
    identb = np.eye(128, dtype=F16)
    identf = np.eye(128, dtype=np.float32)
    maps = []
    for c in range(NCORES):
        maps.append({
            "xt": data["xt"], "idxa": data["idxA"][c],
            "idxb": data["idxB"][c], "apgb": data["apgb"][c],
            "dinvb": data["dinvb"][c],
            "w0": w0, "w1": w1, "w2": w2, "bnp": bnp,
            "identb": identb, "identf": identf,
        })
    return maps


_CACHE = {}


def kernel(x, edge_index, W0, b0, g0, be0, W1, b1, g1, be1, W2, b2, g2, be2,
           _trace=False, _tmpdir=None, _debug=False):
    x = np.asarray(x, np.float32)
    edge_index = np.asarray(edge_index, np.int32)
    args = [np.asarray(a, np.float32)
            for a in (W0, b0, g0, be0, W1, b1, g1, be1, W2, b2, g2, be2)]
    (W0, b0, g0, be0, W1, b1, g1, be1, W2, b2, g2, be2) = args
    # conv bias cancels exactly in training-mode BatchNorm -> ignored.

    key = (edge_index.tobytes()[:256], int(edge_index.sum()), bool(_debug))
    if key not in _CACHE:
        sched, data = preprocess(edge_index, x)
        nc_obj = build_nc(sched, debug=_debug)
        _CACHE[key] = (sched, nc_obj)
    else:
        sched, nc_obj = _CACHE[key]
        _, data = preprocess(edge_index, x)

    in_maps = _make_inmaps(sched, data, W0, W1, W2, g0, be0, g1, be1, g2, be2)
    res = run_bass_kernel_spmd(nc_obj, in_maps, core_ids=list(range(NCORES)),
                               trace=_trace, tmpdir=_tmpdir)

    out = np.zeros((N, DOUT), np.float32)
    for c in range(NCORES):
        o = np.asarray(res.results[c]["out"])
        sn = sched["slot_node"][c]
        real = sn >= 0
        out[sn[real]] = o[real]
    kernel._last_result = res
    kernel._last_sched = sched
    return out



# revision 12
# speedup vs baseline: 1.2435x; 1.2435x over previous
"""GCN 3-layer forward on 8 Trainium2 NeuronCores (Bass/Tile).

Self-contained: hardcodes the problem shapes from the spec.
kernel(**inputs) -> np.ndarray [50000, 128] float32.

Layout: feature-major ("transposed") on chip — features on partitions,
nodes along the free dim.  Nodes are degree-sorted and round-robin
assigned to cores; per-core slots sorted by (A-half degree, B-half
degree) so the segmented reduce is a short list of constant-degree runs
shared by all cores (histograms padded to a common shape).  Message
gather uses dma_gather(transpose=True) from f16 tables in DRAM; the
int16 index limit is handled by an A/B split of the table (cores 0-3
via view [0,32768), cores 4-7 via [TR-32768,TR)).  Layer 1 gathers full
512B rows (elem_size=256 -> [128,2,CHA] out) so each edge costs ONE
SWDGE descriptor on every layer.  Gather streams are packed so no
segment crosses a 896-index sub-call boundary.  The B-phase output is
permuted back to slot order with ap_gather over fp32-paired f16
columns.  Symmetric normalization is factorized: table rows pre-scaled
by dinv[src], aggregates post-scaled by dinv[dst].  Conv bias is
dropped (cancels exactly in training-mode BatchNorm).  BN+LeakyReLU is
one ScalarE activation; BN stats ride accum_out + a tiny AllReduce;
tables are shared with AllGather.
"""
import sys

sys.path.insert(0, "/opt/trn_rl_repo")

import numpy as np
import ml_dtypes

import concourse.bacc as bacc
import concourse.mybir as mybir
import concourse.tile as tile
from concourse.bass_utils import run_bass_kernel_spmd

F16 = np.float16

N, E, DIN, DH, DOUT = 50000, 800000, 128, 256, 128
EPS = 1e-5
SLOPE = 0.01
NCORES = 8
ACORES = 4          # cores 0..3 form the "A" half of the table
CHA = 896           # idxs per gather call
SUB = 4             # gather calls per chunk (one idx-DMA granularity)
CHB = CHA * SUB     # edges per chunk
IMAX = 32768        # int16 index window


# ----------------------------------------------------------------------------
# host-side schedule construction
# ----------------------------------------------------------------------------

def _wrap_idx(arr):
    """1-D int array -> [128, len/16] int16 wrapped+replicated layout."""
    L = len(arr)
    assert L % 16 == 0
    a = np.asarray(arr, np.int16).reshape(L // 16, 16).T  # [16, L/16]
    return np.tile(a, (8, 1)).copy()  # [128, L/16]


def _pack_phase(gvec):
    """Pack even-degree columns into a CHA-subdivided stream.

    gvec: per-column even degrees (bucket-sorted ascending).  Columns are
    packed left-to-right; padding keeps any column's span inside one
    CHA-sized sub-call, and runs split at even column counts (4B output
    alignment).  Returns (runs, colpos, E): runs = [(pos, jcol, n, g)],
    colpos[j] = stream position of column j's first edge, E = padded
    stream length (multiple of CHB).
    """
    M = len(gvec)
    colpos = np.zeros(M, np.int64)
    runs = []
    pos = 0
    j = 0
    while j < M:
        g = int(gvec[j])
        if g == 0:
            j += 1
            continue
        assert 2 * g <= CHA, f"degree {g} too large for sub-call"
        k = j
        while k < M and gvec[k] == g:
            k += 1
        while j < k:
            rem = CHA - (pos % CHA)
            ncols_fit = (rem // g) & ~1
            if ncols_fit < 2:
                pos += rem  # pad to next sub boundary
                continue
            take = min(k - j, ncols_fit)
            runs.append((int(pos), int(j), int(take), int(g)))
            colpos[j:j + take] = pos + np.arange(take) * g
            pos += take * g
            j += take
    Epad = -(-pos // CHB) * CHB
    return runs, colpos, Epad


def _sched_from_runs(runs, Epad):
    """Group runs by chunk; a run never crosses a sub boundary."""
    nchunks = Epad // CHB
    by_chunk = [[] for _ in range(nchunks)]
    for (pos, jcol, n, g) in runs:
        chunk, rem = divmod(pos, CHB)
        sub, off = divmod(rem, CHA)
        assert off + n * g <= CHA
        by_chunk[chunk].append((sub, off, jcol, n, g))
    return by_chunk


def preprocess(edge_index, x):
    src0 = np.asarray(edge_index[0], np.int64)
    dst0 = np.asarray(edge_index[1], np.int64)
    loop = np.arange(N, dtype=np.int64)
    src = np.concatenate([src0, loop])
    dst = np.concatenate([dst0, loop])

    deg = np.bincount(dst, minlength=N)
    dinv = (1.0 / np.sqrt(deg.astype(np.float64))).astype(np.float32)

    order = np.argsort(deg, kind="stable")
    core_of = np.empty(N, np.int32)
    core_of[order] = np.arange(N) % NCORES

    a_mask = core_of[src] < ACORES
    gA = np.bincount(dst[a_mask], minlength=N)
    gB = deg - gA
    gAp = gA + (gA & 1)   # even-padded phase degrees
    gBp = gB + (gB & 1)

    # per-core slot order: sorted by (even A-degree, even B-degree)
    gmaxA = int(gAp.max())
    counts_a = np.zeros((NCORES, gmaxA + 1), np.int64)
    percore_nodes = []
    for c in range(NCORES):
        nodes_c = np.flatnonzero(core_of == c)
        nodes_c = nodes_c[np.lexsort((gBp[nodes_c], gAp[nodes_c]))]
        percore_nodes.append(nodes_c)
        counts_a[c] = np.bincount(gAp[nodes_c], minlength=gmaxA + 1)

    mA = counts_a.max(axis=0)
    mA[0] += 2                        # guarantee pad slots (zero table rows)
    mA += mA & 1                      # even bucket sizes
    nc_raw = int(mA.sum())
    NC = -(-nc_raw // 512) * 512
    mA[0] += NC - nc_raw

    gvecA = np.repeat(np.arange(gmaxA + 1), mA)
    startA = np.concatenate([[0], np.cumsum(mA)])

    slot_node = []
    for c in range(NCORES):
        sn = np.full(NC, -1, np.int64)
        nodes_c = percore_nodes[c]
        gs = gAp[nodes_c]
        ranks = np.arange(len(nodes_c)) - np.searchsorted(gs, gs)
        sn[startA[gs] + ranks] = nodes_c
        slot_node.append(sn)

    slot_of = np.full(N, -1, np.int64)
    for c in range(NCORES):
        real = slot_node[c] >= 0
        slot_of[slot_node[c][real]] = np.flatnonzero(real)

    TR = 2 + NCORES * NC
    baseB = max(0, TR - IMAX)
    row_of = 1 + core_of.astype(np.int64) * NC + slot_of

    assert int(row_of[core_of < ACORES].max()) < IMAX
    assert int(row_of[core_of >= ACORES].min()) >= baseB

    # --- B phase: slots paired (2p, 2p+1); both columns padded to the
    # pair's max even B-degree so ap_gather can move fp32-paired columns.
    gB_slot = np.zeros((NCORES, NC), np.int64)
    for c in range(NCORES):
        real = slot_node[c] >= 0
        gB_slot[c][real] = gBp[slot_node[c][real]]
    pairdeg = gB_slot.reshape(NCORES, NC // 2, 2).max(axis=2)  # [cores, NP]
    NP = NC // 2
    gmaxB = int(pairdeg.max())
    counts_b = np.zeros((NCORES, gmaxB + 1), np.int64)
    for c in range(NCORES):
        counts_b[c] = np.bincount(pairdeg[c], minlength=gmaxB + 1)
    mB = counts_b.max(axis=0)
    npb_raw = int(mB.sum())
    NPB = -(-npb_raw // 16) * 16
    mB[0] += NPB - npb_raw
    NCB = 2 * NPB
    pairdegvec = np.repeat(np.arange(gmaxB + 1), mB)   # per B-pair degree
    gvecB = np.repeat(pairdegvec, 2)                   # per B-column degree
    startBp = np.concatenate([[0], np.cumsum(mB)])

    paircol = np.zeros((NCORES, NP), np.int64)  # slot-pair -> B-pair pos
    for c in range(NCORES):
        pd = pairdeg[c]
        o = np.argsort(pd, kind="stable")
        ranks = np.arange(NP) - np.searchsorted(pd[o], pd[o])
        pc = np.empty(NP, np.int64)
        pc[o] = startBp[pd[o]] + ranks
        paircol[c] = pc

    zA = int(startA[1])                   # row of (core 0, slot m0A-1)
    zB = int(7 * NC + startA[1])          # row of (core 7, slot m0A-1)
    runsA, colposA, EA = _pack_phase(gvecA)
    runsB, colposB, EB = _pack_phase(gvecB)
    chunksA = _sched_from_runs(runsA, EA)
    chunksB = _sched_from_runs(runsB, EB)

    idxA, idxB, apgb_l, dinvb_l = [], [], [], []
    for c in range(NCORES):
        # --- A stream (default idx = zero row)
        sA = np.full(EA, zA, np.int64)
        m = (core_of[dst] == c) & a_mask
        es, cols = src[m], slot_of[dst[m]]
        o = np.argsort(cols, kind="stable")
        es, cols = es[o], cols[o]
        ranks = np.arange(len(cols)) - np.searchsorted(cols, cols)
        sA[colposA[cols] + ranks] = row_of[es]
        assert sA.max() < IMAX
        idxA.append(_wrap_idx(sA))

        # --- B stream (default = B zero row)
        sB = np.full(EB, zB - baseB, np.int64)
        m = (core_of[dst] == c) & (~a_mask)
        es, sl = src[m], slot_of[dst[m]]
        cols = paircol[c][sl // 2] * 2 + (sl & 1)
        o = np.argsort(cols, kind="stable")
        es, cols = es[o], cols[o]
        ranks = np.arange(len(cols)) - np.searchsorted(cols, cols)
        sB[colposB[cols] + ranks] = row_of[es] - baseB
        assert sB.min() >= 0 and sB.max() < IMAX
        idxB.append(_wrap_idx(sB))

        apgb_l.append(_wrap_idx(paircol[c]))

        dv = np.zeros(NC, np.float32)
        real = slot_node[c] >= 0
        dv[real] = dinv[slot_node[c][real]]
        dinvb_l.append(np.tile(dv[None, :].astype(F16), (128, 1)))

    xt = np.zeros((TR, DIN), F16)
    xs = (np.asarray(x, np.float32) * dinv[:, None]).astype(F16)
    xt[row_of] = xs

    sched = {
        "NC": NC, "NCB": NCB, "NPB": NPB, "TR": TR, "baseB": baseB,
        "chunksA": chunksA, "chunksB": chunksB, "EA": EA, "EB": EB,
        "m0A": int(startA[1]), "m0B": int(2 * startBp[1]),
        "slot_node": slot_node,
    }
    data = {"xt": xt, "idxA": idxA, "idxB": idxB, "apgb": apgb_l,
            "dinvb": dinvb_l}
    return sched, data


# ----------------------------------------------------------------------------
# device kernel builder
# ----------------------------------------------------------------------------

def build_nc(sched, debug=False):
    NC, NCB, TR = sched["NC"], sched["NCB"], sched["TR"]
    NPB = sched["NPB"]
    baseB = sched["baseB"]
    EA, EB = sched["EA"], sched["EB"]
    fp32 = mybir.dt.float32
    f16 = mybir.dt.float16
    i16 = mybir.dt.int16
    AF = mybir.ActivationFunctionType
    OP = mybir.AluOpType
    NCH = NC // 512

    nc = bacc.Bacc("TRN2", target_bir_lowering=False, num_devices=NCORES,
                   num_swdge_queues=4, dynamic_dma_scratch_size=30720)

    xt_d = nc.dram_tensor("xt", [TR, DIN], f16, kind="ExternalInput")
    idxa_d = nc.dram_tensor("idxa", [128, EA // 16], i16, kind="ExternalInput")
    idxb_d = nc.dram_tensor("idxb", [128, EB // 16], i16, kind="ExternalInput")
    apgb_d = nc.dram_tensor("apgb", [128, NC // 32], i16, kind="ExternalInput")
    dinvb_d = nc.dram_tensor("dinvb", [128, NC], f16, kind="ExternalInput")
    w0_d = nc.dram_tensor("w0", [128, 256], f16, kind="ExternalInput")
    w1_d = nc.dram_tensor("w1", [128, 512], f16, kind="ExternalInput")
    w2_d = nc.dram_tensor("w2", [128, 256], f16, kind="ExternalInput")
    bnp_d = nc.dram_tensor("bnp", [128, 10], fp32, kind="ExternalInput")
    identb_d = nc.dram_tensor("identb", [128, 128], f16, kind="ExternalInput")
    identf_d = nc.dram_tensor("identf", [128, 128], fp32, kind="ExternalInput")
    out_d = nc.dram_tensor("out", [NC, DOUT], fp32, kind="ExternalOutput")
    dbg = {}
    if debug:
        for name, shape in [
            ("dbg_outA0", [128, NC]), ("dbg_outB0", [128, NCB]),
            ("dbg_agg0", [128, NC]), ("dbg_h1", [128, 2 * NC]),
            ("dbg_agg1", [128, 2 * NC]), ("dbg_outA1", [128, 2 * NC]),
            ("dbg_outB1", [128, 2 * NCB]),
        ]:
            dbg[name] = nc.dram_tensor(name, shape, fp32,
                                       kind="ExternalOutput")

    with tile.TileContext(nc) as tc:
        with (
            tc.tile_pool(name="const", bufs=1) as constp,
            tc.tile_pool(name="gat", bufs=2) as gatp,
            tc.tile_pool(name="red", bufs=1) as redp,
            tc.tile_pool(name="agg", bufs=2) as aggp,
            tc.tile_pool(name="perm", bufs=1) as permp,
            tc.tile_pool(name="small", bufs=2) as smallp,
            tc.tile_pool(name="rowt", bufs=2) as rowp,
            tc.tile_pool(name="ps", bufs=3, space="PSUM") as psp,
            tc.tile_pool(name="pst", bufs=2, space="PSUM") as pstp,
            tc.tile_pool(name="pstf", bufs=2, space="PSUM") as pstfp,
            tc.tile_pool(name="dram", bufs=1, space="DRAM") as dramp,
        ):
            # ---- resident constants
            apgb = constp.tile([128, NC // 32], i16, tag="apgb")
            dinvb = constp.tile([128, NC], f16, tag="dinvb")
            w0 = constp.tile([128, 256], f16, tag="w0")
            w1 = constp.tile([128, 512], f16, tag="w1")
            w2 = constp.tile([128, 256], f16, tag="w2")
            bnp = constp.tile([128, 10], fp32, tag="bnp")
            identb = constp.tile([128, 128], f16, tag="identb")
            identf = constp.tile([128, 128], fp32, tag="identf")
            for t, d in [(apgb, apgb_d),
                         (dinvb, dinvb_d), (w0, w0_d), (w1, w1_d),
                         (w2, w2_d), (bnp, bnp_d), (identb, identb_d),
                         (identf, identf_d)]:
                nc.sync.dma_start(t[:], d[:])

            tbl = dramp.tile([TR, DH], f16, tag="tbl", addr_space="Shared")
            tbl2 = dramp.tile([TR, DOUT], f16, tag="tbl2",
                              addr_space="Shared")
            agsrc = dramp.tile([NC, DH], f16, tag="agsrc")
            agsrc2 = dramp.tile([NC, DOUT], f16, tag="agsrc2")

            def gather_reduce(table, elem, blocks):
                """A+B gather phases -> (outA f16, outB f16) [128,2,*].

                elem: f16 elements per table row (128 or 256); a single
                gather call fetches the whole row (one descriptor/edge)."""
                outA = redp.tile([128, 2, NC], f16, tag="outA")
                outB = redp.tile([128, 2, NCB], f16, tag="outB")
                with nc.allow_low_precision(reason="DVE accumulates fp32"):
                    for phase in ("A", "B"):
                        if phase == "A":
                            idxd, chunks, outX, m0 = \
                                idxa_d, sched["chunksA"], outA, sched["m0A"]
                            view = table[0:min(TR, IMAX), :]
                        else:
                            idxd, chunks, outX, m0 = \
                                idxb_d, sched["chunksB"], outB, sched["m0B"]
                            view = table[baseB:TR, :]
                        for j in range(blocks):
                            if m0 > 0:
                                nc.vector.memset(outX[:, j, :m0], 0)
                        for k, chk in enumerate(chunks):
                            gbs = [gatp.tile([128, 2, CHA], f16,
                                             tag=f"gs{sx}", name=f"gs{sx}")
                                   for sx in range(SUB)]
                            idxt = gatp.tile([128, CHB // 16], i16,
                                             tag="idxt")
                            nc.sync.dma_start(
                                idxt[:], idxd[:, k * (CHB // 16):
                                              (k + 1) * (CHB // 16)])
                            for sx in range(SUB):
                                nc.gpsimd.dma_gather(
                                    out_ap=gbs[sx][:, :blocks, :],
                                    in_ap=view,
                                    idxs_ap=idxt[:, sx * (CHA // 16):
                                                 (sx + 1) * (CHA // 16)],
                                    num_idxs=CHA,
                                    num_idxs_reg=CHA,
                                    elem_size=elem,
                                    transpose=True,
                                    queue_num=sx % 4,
                                )
                            for (sub, off, ocol, n, g) in chk:
                                for j in range(blocks):
                                    nc.vector.tensor_reduce(
                                        outX[:, j, ocol:ocol + n],
                                        gbs[sub][:, j, off:off + n * g]
                                        .rearrange("p (n g) -> p n g", g=g),
                                        axis=mybir.AxisListType.X,
                                        op=OP.add,
                                    )
                return outA, outB

            def merge(outA, outB, blocks):
                """B-perm + add + dinv[dst] scale -> aggT f16 [128,2,NC]."""
                aggT = aggp.tile([128, 2, NC], f16, tag="aggbuf")
                for j in range(blocks):
                    tmp = permp.tile([128, NC // 2], fp32, tag="ptmp")
                    nc.gpsimd.ap_gather(
                        out_ap=tmp[:],
                        in_ap=outB[:, j, :].bitcast(fp32),
                        idxs_ap=apgb[:],
                        channels=128,
                        num_elems=NPB,
                        d=1,
                        num_idxs=NC // 2,
                    )
                    tb = tmp[:].bitcast(f16)
                    nc.vector.tensor_tensor(aggT[:, j, :], tb,
                                            outA[:, j, :], OP.add)
                    nc.vector.tensor_tensor(aggT[:, j, :], aggT[:, j, :],
                                            dinvb[:], OP.mult)
                return aggT

            def bn_consts(st, blocks_out, bn_off, layer):
                """AllReduce stats -> per-feature scale A / bias B tiles."""
                stin = dramp.tile([128, 4], fp32, tag=f"stin{layer}")
                stout = dramp.tile([128, 4], fp32, tag=f"stout{layer}",
                                   addr_space="Shared")
                nc.gpsimd.dma_start(stin[:], st[:])
                nc.gpsimd.collective_compute(
                    "AllReduce", OP.add,
                    replica_groups=[list(range(NCORES))],
                    ins=[stin.opt()], outs=[stout.opt()],
                )
                stg = smallp.tile([128, 4], fp32, tag="stg")
                nc.sync.dma_start(stg[:], stout[:])
                b = blocks_out
                mu = smallp.tile([128, 2], fp32, tag="mu")
                va = smallp.tile([128, 2], fp32, tag="va")
                Ab = smallp.tile([128, 2], fp32, tag="Ab")
                Bb = smallp.tile([128, 2], fp32, tag="Bb")
                musq = smallp.tile([128, 2], fp32, tag="musq")
                rstd = smallp.tile([128, 2], fp32, tag="rstd")
                nc.vector.tensor_scalar(mu[:, :b], stg[:, 0:b], 1.0 / N, None,
                                        op0=OP.mult)
                nc.vector.tensor_scalar(va[:, :b], stg[:, 2:2 + b], 1.0 / N,
                                        None, op0=OP.mult)
                nc.vector.tensor_tensor(musq[:, :b], mu[:, :b], mu[:, :b],
                                        OP.mult)
                nc.vector.tensor_tensor(va[:, :b], va[:, :b], musq[:, :b],
                                        OP.subtract)
                sqv = smallp.tile([128, 2], fp32, tag="sqv")
                nc.vector.tensor_scalar(sqv[:, :b], va[:, :b], EPS, None,
                                        op0=OP.add)
                nc.scalar.activation(sqv[:, :b], sqv[:, :b], AF.Sqrt)
                nc.vector.reciprocal(rstd[:, :b], sqv[:, :b])
                gsl = bnp[:, bn_off:bn_off + b]
                bsl = bnp[:, bn_off + b:bn_off + 2 * b]
                nc.vector.tensor_tensor(Ab[:, :b], rstd[:, :b], gsl, OP.mult)
                nc.vector.tensor_tensor(Bb[:, :b], mu[:, :b], Ab[:, :b],
                                        OP.mult)
                nc.vector.tensor_tensor(Bb[:, :b], bsl, Bb[:, :b],
                                        OP.subtract)
                return Ab, Bb

            def conv_bn(aggT, wt, KT, bn_off, lrelu, layer):
                """matmul (out 2 blocks of 128) + BN(+lrelu) -> h f16."""
                cv = aggp.tile([128, 2, NC], f16, tag="aggbuf")
                ssum = smallp.tile([128, 2, NCH], fp32, tag="ssum")
                sqsum = smallp.tile([128, 2, NCH], fp32, tag="sqsum")
                for j in range(2):
                    for t in range(NCH):
                        ps = psp.tile([128, 512], fp32, tag="cps")
                        sl = slice(t * 512, (t + 1) * 512)
                        for kt in range(KT):
                            lhsT = wt[:, kt * 256 + j * 128:
                                      kt * 256 + (j + 1) * 128]
                            nc.tensor.matmul(ps[:], lhsT, aggT[:, kt, sl],
                                             start=(kt == 0),
                                             stop=(kt == KT - 1))
                        sq = smallp.tile([128, 512], f16, tag="sqd")
                        nc.scalar.activation(cv[:, j, sl], ps[:], AF.Copy,
                                             accum_out=ssum[:, j, t:t + 1])
                        nc.scalar.activation(sq[:], ps[:], AF.Square,
                                             accum_out=sqsum[:, j, t:t + 1])
                st = smallp.tile([128, 4], fp32, tag="stl")
                for j in range(2):
                    nc.vector.tensor_reduce(st[:, j:j + 1], ssum[:, j, :],
                                            axis=mybir.AxisListType.X,
                                            op=OP.add)
                    nc.vector.tensor_reduce(st[:, 2 + j:3 + j], sqsum[:, j, :],
                                            axis=mybir.AxisListType.X,
                                            op=OP.add)
                Ab, Bb = bn_consts(st, 2, bn_off, layer)
                h = aggp.tile([128, 2, NC], f16, tag="aggbuf")
                fn = AF.Lrelu if lrelu else AF.Identity
                for j in range(2):
                    nc.scalar.activation(h[:, j, :], cv[:, j, :], fn,
                                         bias=Bb[:, j:j + 1],
                                         scale=Ab[:, j:j + 1], alpha=SLOPE)
                return h, cv, st

            def write_rows(srcT, blocks, dst_dram, width, prescale):
                """(optional dinv[src] prescale) + transpose + DMA rows."""
                if prescale:
                    hs = aggp.tile([128, 2, NC], f16, tag="aggbuf")
                    for j in range(blocks):
                        nc.vector.tensor_tensor(hs[:, j, :], srcT[:, j, :],
                                                dinvb[:], OP.mult)
                    srcT = hs
                for t in range(NC // 128):
                    row = rowp.tile([128, width], f16, tag="rowt")
                    for j in range(blocks):
                        pt = pstp.tile([128, 128], f16, tag="tps")
                        nc.tensor.transpose(
                            pt[:], srcT[:, j, t * 128:(t + 1) * 128],
                            identb[:])
                        nc.scalar.activation(row[:, j * 128:(j + 1) * 128],
                                             pt[:], AF.Copy)
                    nc.sync.dma_start(dst_dram[t * 128:(t + 1) * 128, :],
                                      row[:])
                return srcT

            # ================= layer 0 =================
            outA, outB = gather_reduce(xt_d, DIN, 1)
            if debug:
                nc.gpsimd.dma_start(dbg["dbg_outA0"][:], outA[:, 0, :])
                nc.gpsimd.dma_start(dbg["dbg_outB0"][:], outB[:, 0, :])
            aggT = merge(outA, outB, 1)
            if debug:
                nc.gpsimd.dma_start(dbg["dbg_agg0"][:], aggT[:, 0, :])
            h1, cv0, st0 = conv_bn(aggT, w0, 1, 0, True, 0)
            if debug:
                nc.gpsimd.dma_start(
                    dbg["dbg_h1"][:].rearrange("p (a b) -> p a b", a=2),
                    h1[:])
            write_rows(h1, 2, agsrc, DH, prescale=True)
            nc.gpsimd.collective_compute(
                "AllGather", OP.bypass,
                replica_groups=[list(range(NCORES))],
                ins=[agsrc.opt()], outs=[tbl[1:1 + NCORES * NC, :]],
            )

            # ================= layer 1 =================
            outA, outB = gather_reduce(tbl, DH, 2)
            if debug:
                nc.gpsimd.dma_start(
                    dbg["dbg_outA1"][:].rearrange("p (a b) -> p a b", a=2),
                    outA[:])
                nc.gpsimd.dma_start(
                    dbg["dbg_outB1"][:].rearrange("p (a b) -> p a b", a=2),
                    outB[:])
            aggT = merge(outA, outB, 2)
            if debug:
                nc.gpsimd.dma_start(
                    dbg["dbg_agg1"][:].rearrange("p (a b) -> p a b", a=2),
                    aggT[:])
            h2, _, _ = conv_bn(aggT, w1, 2, 4, True, 1)
            # transform-first for layer 2: T2 = W2 @ (dinv * h2)
            hs2 = aggp.tile([128, 2, NC], f16, tag="aggbuf")
            for j in range(2):
                nc.vector.tensor_tensor(hs2[:, j, :], h2[:, j, :], dinvb[:],
                                        OP.mult)
            t2 = aggp.tile([128, 2, NC], f16, tag="aggbuf")
            for t in range(NCH):
                ps = psp.tile([128, 512], fp32, tag="cps")
                sl = slice(t * 512, (t + 1) * 512)
                for kt in range(2):
                    nc.tensor.matmul(ps[:], w2[:, kt * 128:(kt + 1) * 128],
                                     hs2[:, kt, sl],
                                     start=(kt == 0), stop=(kt == 1))
                nc.scalar.activation(t2[:, 0, sl], ps[:], AF.Copy)
            write_rows(t2, 1, agsrc2, DOUT, prescale=False)
            nc.gpsimd.collective_compute(
                "AllGather", OP.bypass,
                replica_groups=[list(range(NCORES))],
                ins=[agsrc2.opt()], outs=[tbl2[1:1 + NCORES * NC, :]],
            )

            # ================= layer 2 =================
            outA, outB = gather_reduce(tbl2, DOUT, 1)
            aggT = merge(outA, outB, 1)
            # aggT IS the conv output (transform-first); BN only, no lrelu.
            ssum = smallp.tile([128, 2, NCH], fp32, tag="ssum")
            sqsum = smallp.tile([128, 2, NCH], fp32, tag="sqsum")
            for t in range(NCH):
                sl = slice(t * 512, (t + 1) * 512)
                sq = smallp.tile([128, 512], f16, tag="sqd")
                nc.scalar.activation(sq[:], aggT[:, 0, sl], AF.Square,
                                     accum_out=sqsum[:, 0, t:t + 1])
                nc.vector.tensor_reduce(ssum[:, 0, t:t + 1], aggT[:, 0, sl],
                                        axis=mybir.AxisListType.X, op=OP.add)
            st = smallp.tile([128, 4], fp32, tag="stl")
            nc.vector.tensor_reduce(st[:, 0:1], ssum[:, 0, :],
                                    axis=mybir.AxisListType.X, op=OP.add)
            nc.vector.tensor_reduce(st[:, 2:3], sqsum[:, 0, :],
                                    axis=mybir.AxisListType.X, op=OP.add)
            nc.vector.memset(st[:, 1:2], 0)
            nc.vector.memset(st[:, 3:4], 0)
            Ab, Bb = bn_consts(st, 1, 8, 2)
            for t in range(NC // 128):
                hf = smallp.tile([128, 128], fp32, tag="hfin")
                nc.scalar.activation(hf[:], aggT[:, 0, t * 128:(t + 1) * 128],
                                     AF.Identity,
                                     bias=Bb[:, 0:1], scale=Ab[:, 0:1])
                row = rowp.tile([128, DOUT], fp32, tag="rowtf")
                pt = pstfp.tile([128, 128], fp32, tag="tpsf")
                nc.tensor.transpose(pt[:], hf[:], identf[:])
                nc.vector.tensor_copy(row[:], pt[:])
                nc.sync.dma_start(out_d[t * 128:(t + 1) * 128, :], row[:])

    nc.compile()
    return nc


# ----------------------------------------------------------------------------
# entry point
# ----------------------------------------------------------------------------

def _make_inmaps(sched, data, W0, W1, W2, g0, be0, g1, be1, g2, be2):
    w0 = np.ascontiguousarray(W0.T.astype(F16))
    w1 = np.ascontiguousarray(
        W1.T.reshape(2, 128, 256).transpose(1, 0, 2).reshape(128, 512)
        .astype(F16))
    w2 = np.ascontiguousarray(
        W2.T.reshape(2, 128, 128).transpose(1, 0, 2).reshape(128, 256)
        .astype(F16))
    bnp = np.zeros((128, 10), np.float32)
    bnp[:, 0:2] = g0.reshape(2, 128).T
    bnp[:, 2:4] = be0.reshape(2, 128).T
    bnp[:, 4:6] = g1.reshape(2, 128).T
    bnp[:, 6:8] = be1.reshape(2, 128).T
    bnp[:, 8] = g2
    bnp[:, 9] = be2
    identb = np.eye(128, dtype=F16)
    identf = np.eye(128, dtype=np.float32)
    maps = []
    for c in range(NCORES):
        maps.append({
            "xt": data["xt"], "idxa": data["idxA"][c],
            "idxb": data["idxB"][c], "apgb": data["apgb"][c],
            "dinvb": data["dinvb"][c],
            "w0": w0, "w1": w1, "w2": w2, "bnp": bnp,
            "identb": identb, "identf": identf,
        })
    return maps


_CACHE = {}


def kernel(x, edge_index, W0, b0, g0, be0, W1, b1, g1, be1, W2, b2, g2, be2,
           _trace=False, _tmpdir=None, _debug=False):
    x = np.asarray(x, np.float32)
    edge_index = np.asarray(edge_index, np.int32)
    args = [np.asarray(a, np.float32)
            for a in (W0, b0, g0, be0, W1, b1, g1, be1, W2, b2, g2, be2)]
    (W0, b0, g0, be0, W1, b1, g1, be1, W2, b2, g2, be2) = args
    # conv bias cancels exactly in training-mode BatchNorm -> ignored.

    key = (edge_index.tobytes()[:256], int(edge_index.sum()), bool(_debug))
    if key not in _CACHE:
        sched, data = preprocess(edge_index, x)
        nc_obj = build_nc(sched, debug=_debug)
        _CACHE[key] = (sched, nc_obj)
    else:
        sched, nc_obj = _CACHE[key]
        _, data = preprocess(edge_index, x)

    in_maps = _make_inmaps(sched, data, W0, W1, W2, g0, be0, g1, be1, g2, be2)
    res = run_bass_kernel_spmd(nc_obj, in_maps, core_ids=list(range(NCORES)),
                               trace=_trace, tmpdir=_tmpdir)

    out = np.zeros((N, DOUT), np.float32)
    for c in range(NCORES):
        o = np.asarray(res.results[c]["out"])
        sn = sched["slot_node"][c]
        real = sn >= 0
        out[sn[real]] = o[real]
    kernel._last_result = res
    kernel._last_sched = sched
    return out


# revision 18
# speedup vs baseline: 2.1620x; 1.7386x over previous
"""GCN 3-layer forward on 8 Trainium2 NeuronCores (Bass/Tile).

Self-contained: hardcodes the problem shapes from the spec.
kernel(**inputs) -> np.ndarray [50000, 128] float32.

Layout: feature-major ("transposed") on chip — features on partitions,
nodes along the free dim.  Nodes are degree-sorted and round-robin
assigned to cores; per-core slots sorted by (A-half degree, B-half
degree) so the segmented reduce is a short list of constant-degree runs
shared by all cores (histograms padded to a common shape).  Message
gather uses dma_gather(transpose=True) from f16 tables in DRAM; the
int16 index limit is handled by an A/B split of the table (cores 0-3
via view [0,32768), cores 4-7 via [TR-32768,TR)).  Layer 1 gathers full
512B rows (elem_size=256 -> [128,2,CHA] out) so each edge costs ONE
SWDGE descriptor on every layer.  Gather streams are packed so no
segment crosses a 896-index sub-call boundary.  The B-phase output is
permuted back to slot order with ap_gather over fp32-paired f16
columns.  Symmetric normalization is factorized: table rows pre-scaled
by dinv[src], aggregates post-scaled by dinv[dst].  Conv bias is
dropped (cancels exactly in training-mode BatchNorm).  BN+LeakyReLU is
one ScalarE activation; BN stats ride accum_out + a tiny AllReduce;
tables are shared with AllGather.
"""
import sys

sys.path.insert(0, "/opt/trn_rl_repo")

import numpy as np
import ml_dtypes

import concourse.bacc as bacc
import concourse.mybir as mybir
import concourse.tile as tile
from concourse.bass_utils import run_bass_kernel_spmd

F16 = np.float16

N, E, DIN, DH, DOUT = 50000, 800000, 128, 256, 128
EPS = 1e-5
SLOPE = 0.01
NCORES = 8
ACORES = 4          # cores 0..3 form the "A" half of the table
CHA = 896           # idxs per gather call
SUB = 4             # gather calls per chunk (one idx-DMA granularity)
CHB = CHA * SUB     # edges per chunk
IMAX = 32768        # int16 index window


# ----------------------------------------------------------------------------
# host-side schedule construction
# ----------------------------------------------------------------------------

def _wrap_idx(arr):
    """1-D int array -> [128, len/16] int16 wrapped+replicated layout."""
    L = len(arr)
    assert L % 16 == 0
    a = np.asarray(arr, np.int16).reshape(L // 16, 16).T  # [16, L/16]
    return np.tile(a, (8, 1)).copy()  # [128, L/16]


def _pack_phase(gvec):
    """Pack even-degree columns into a CHA-subdivided stream.

    gvec: per-column even degrees (bucket-sorted ascending).  Columns are
    packed left-to-right; padding keeps any column's span inside one
    CHA-sized sub-call, and runs split at even column counts (4B output
    alignment).  Returns (runs, colpos, E): runs = [(pos, jcol, n, g)],
    colpos[j] = stream position of column j's first edge, E = padded
    stream length (multiple of CHB).
    """
    M = len(gvec)
    colpos = np.zeros(M, np.int64)
    runs = []
    pos = 0
    j = 0
    while j < M:
        g = int(gvec[j])
        if g == 0:
            j += 1
            continue
        assert 2 * g <= CHA, f"degree {g} too large for sub-call"
        k = j
        while k < M and gvec[k] == g:
            k += 1
        while j < k:
            rem = CHA - (pos % CHA)
            ncols_fit = (rem // g) & ~1
            if ncols_fit < 2:
                pos += rem  # pad to next sub boundary
                continue
            take = min(k - j, ncols_fit)
            runs.append((int(pos), int(j), int(take), int(g)))
            colpos[j:j + take] = pos + np.arange(take) * g
            pos += take * g
            j += take
    Epad = -(-pos // CHB) * CHB
    return runs, colpos, Epad


def _sched_from_runs(runs, Epad):
    """Group runs by chunk; a run never crosses a sub boundary."""
    nchunks = Epad // CHB
    by_chunk = [[] for _ in range(nchunks)]
    for (pos, jcol, n, g) in runs:
        chunk, rem = divmod(pos, CHB)
        sub, off = divmod(rem, CHA)
        assert off + n * g <= CHA
        by_chunk[chunk].append((sub, off, jcol, n, g))
    return by_chunk


def _balance_sides(src, dst, rounds=60, seed=0):
    """Side (+1=A, -1=B) per node so each dst's in-edges split ~evenly
    (gA = ceil(g/2) target) and |A| == N/2.  Greedy local search."""
    g = np.bincount(dst, minlength=N)
    target = (g & 1).astype(np.int64)
    k_out = np.bincount(src, minlength=N).astype(np.int64)
    o = np.argsort(g, kind="stable")
    s = np.empty(N, np.int64)
    s[o] = np.where(np.arange(N) % 2 == 0, 1, -1)
    rng = np.random.default_rng(seed)
    best_err, best_s = None, s.copy()
    for _ in range(rounds):
        dev = np.bincount(dst, weights=s[src].astype(np.float64),
                          minlength=N).astype(np.int64) - target
        err = int(np.abs(dev).sum())
        if best_err is None or err < best_err:
            best_err, best_s = err, s.copy()
        if err == 0:
            break
        W = np.bincount(src, weights=dev[dst].astype(np.float64),
                        minlength=N).astype(np.int64)
        gain = s * W - k_out
        candA = np.flatnonzero((s > 0) & (gain > 0))
        candB = np.flatnonzero((s < 0) & (gain > 0))
        m = min(len(candA), len(candB))
        if m == 0:
            break
        m = max(1, int(m * 0.35))
        s[rng.choice(candA, m, replace=False)] *= -1
        s[rng.choice(candB, m, replace=False)] *= -1
    return best_s


def preprocess(edge_index, x):
    src0 = np.asarray(edge_index[0], np.int64)
    dst0 = np.asarray(edge_index[1], np.int64)
    loop = np.arange(N, dtype=np.int64)
    src = np.concatenate([src0, loop])
    dst = np.concatenate([dst0, loop])

    deg = np.bincount(dst, minlength=N)
    dinv = (1.0 / np.sqrt(deg.astype(np.float64))).astype(np.float32)

    sides = _balance_sides(src, dst)
    a_mask = sides[src] > 0
    gA = np.bincount(dst[a_mask], minlength=N)
    gB = deg - gA
    gAp = gA + (gA & 1)   # even-padded phase degrees
    gBp = gB + (gB & 1)

    # core assignment: within each side, (gAp,gBp)-sorted round robin
    core_of = np.empty(N, np.int64)
    for sd, base in [(1, 0), (-1, 4)]:
        nodes = np.flatnonzero(sides == sd)
        o2 = nodes[np.lexsort((gBp[nodes], gAp[nodes]))]
        core_of[o2] = base + np.arange(len(o2)) % 4

    cnts = [int((core_of == c).sum()) for c in range(NCORES)]
    NC = -(-(2 + max(cnts)) // 512) * 512
    assert 4 * NC < IMAX

    # slots 0,1 reserved as zero rows; nodes at slots 2.. in lexsort order
    slot_node = []
    avec = np.zeros(NC, np.int64)
    bvec = np.zeros(NC, np.int64)
    for c in range(NCORES):
        nodes_c = np.flatnonzero(core_of == c)
        nodes_c = nodes_c[np.lexsort((gBp[nodes_c], gAp[nodes_c]))]
        sn = np.full(NC, -1, np.int64)
        sn[2:2 + len(nodes_c)] = nodes_c
        slot_node.append(sn)
        La = np.zeros(NC, np.int64)
        Lb = np.zeros(NC, np.int64)
        La[2:2 + len(nodes_c)] = gAp[nodes_c]
        Lb[2:2 + len(nodes_c)] = gBp[nodes_c]
        avec = np.maximum(avec, La)
        bvec = np.maximum(bvec, Lb)
    avec = np.maximum(avec, 2)   # no-memset: every slot reduces >= 2 entries
    bvec = np.maximum(bvec, 2)

    slot_of = np.full(N, -1, np.int64)
    for c in range(NCORES):
        real = slot_node[c] >= 0
        slot_of[slot_node[c][real]] = np.flatnonzero(real)

    TR = 2 + NCORES * NC
    baseB = max(0, TR - IMAX)
    row_of = 1 + core_of * NC + slot_of

    assert int(row_of[core_of < ACORES].max()) < IMAX
    assert int(row_of[core_of >= ACORES].min()) >= baseB

    zA = 1                      # core 0, slot 0 (always a zero row)
    zB = 1 + 7 * NC             # core 7, slot 0
    runsA, colposA, EA = _pack_phase(avec)
    runsB, colposB, EB = _pack_phase(bvec)
    chunksA = _sched_from_runs(runsA, EA)
    chunksB = _sched_from_runs(runsB, EB)

    idxA, idxB, dinvb_l, dinvt_l = [], [], [], []
    for c in range(NCORES):
        # --- A stream (default idx = zero row)
        sA = np.full(EA, zA, np.int64)
        m = (core_of[dst] == c) & a_mask
        es, cols = src[m], slot_of[dst[m]]
        o = np.argsort(cols, kind="stable")
        es, cols = es[o], cols[o]
        ranks = np.arange(len(cols)) - np.searchsorted(cols, cols)
        sA[colposA[cols] + ranks] = row_of[es]
        assert sA.max() < IMAX
        idxA.append(_wrap_idx(sA))

        # --- B stream (default = B zero row), direct slot columns
        sB = np.full(EB, zB - baseB, np.int64)
        m = (core_of[dst] == c) & (~a_mask)
        es, cols = src[m], slot_of[dst[m]]
        o = np.argsort(cols, kind="stable")
        es, cols = es[o], cols[o]
        ranks = np.arange(len(cols)) - np.searchsorted(cols, cols)
        sB[colposB[cols] + ranks] = row_of[es] - baseB
        assert sB.min() >= 0 and sB.max() < IMAX
        idxB.append(_wrap_idx(sB))

        dv = np.zeros(NC, np.float32)
        real = slot_node[c] >= 0
        dv[real] = dinv[slot_node[c][real]]
        dinvb_l.append(np.tile(dv[None, :].astype(F16), (128, 1)))
        dinvt_l.append(np.ascontiguousarray(
            dv.reshape(NC // 128, 128).T.astype(np.float32)))

    xt = np.zeros((TR, DIN), F16)
    xs = (np.asarray(x, np.float32) * dinv[:, None]).astype(F16)
    xt[row_of] = xs

    sched = {
        "NC": NC, "TR": TR, "baseB": baseB,
        "chunksA": chunksA, "chunksB": chunksB, "EA": EA, "EB": EB,
        "slot_node": slot_node,
    }
    data = {"xt": xt, "idxA": idxA, "idxB": idxB,
            "dinvb": dinvb_l, "dinvt": dinvt_l}
    return sched, data


# ----------------------------------------------------------------------------
# device kernel builder
# ----------------------------------------------------------------------------

def build_nc(sched, debug=False):
    NC, TR = sched["NC"], sched["TR"]
    baseB = sched["baseB"]
    EA, EB = sched["EA"], sched["EB"]
    fp32 = mybir.dt.float32
    f16 = mybir.dt.float16
    i16 = mybir.dt.int16
    AF = mybir.ActivationFunctionType
    OP = mybir.AluOpType
    NCH = NC // 512

    nc = bacc.Bacc("TRN2", target_bir_lowering=False, num_devices=NCORES,
                   num_swdge_queues=4, dynamic_dma_scratch_size=16384)

    xt_d = nc.dram_tensor("xt", [TR, DIN], f16, kind="ExternalInput")
    idxa_d = nc.dram_tensor("idxa", [128, EA // 16], i16, kind="ExternalInput")
    idxb_d = nc.dram_tensor("idxb", [128, EB // 16], i16, kind="ExternalInput")
    dinvb_d = nc.dram_tensor("dinvb", [128, NC], f16, kind="ExternalInput")
    dinvt_d = nc.dram_tensor("dinvt", [128, NC // 128], fp32,
                             kind="ExternalInput")
    w0_d = nc.dram_tensor("w0", [128, 256], f16, kind="ExternalInput")
    w1_d = nc.dram_tensor("w1", [128, 512], f16, kind="ExternalInput")
    w2_d = nc.dram_tensor("w2", [128, 256], f16, kind="ExternalInput")
    bnp_d = nc.dram_tensor("bnp", [128, 10], fp32, kind="ExternalInput")
    identb_d = nc.dram_tensor("identb", [128, 128], f16, kind="ExternalInput")
    identf_d = nc.dram_tensor("identf", [128, 128], fp32, kind="ExternalInput")
    out_d = nc.dram_tensor("out", [NC, DOUT], fp32, kind="ExternalOutput")
    dbg = {}
    if debug:
        for name, shape in [
            ("dbg_outA0", [128, NC]), ("dbg_outB0", [128, NC]),
            ("dbg_agg0", [128, NC]), ("dbg_h1", [128, 2 * NC]),
            ("dbg_agg1", [128, 2 * NC]), ("dbg_outA1", [128, 2 * NC]),
            ("dbg_outB1", [128, 2 * NC]),
        ]:
            dbg[name] = nc.dram_tensor(name, shape, fp32,
                                       kind="ExternalOutput")

    with tile.TileContext(nc) as tc:
        with (
            tc.tile_pool(name="const", bufs=1) as constp,
            tc.tile_pool(name="gat", bufs=3) as gatp,
            tc.tile_pool(name="gix", bufs=3) as gixp,
            tc.tile_pool(name="red", bufs=1) as redp,
            tc.tile_pool(name="agg", bufs=2) as aggp,
            tc.tile_pool(name="small", bufs=2) as smallp,
            tc.tile_pool(name="rowt", bufs=2) as rowp,
            tc.tile_pool(name="ps", bufs=3, space="PSUM") as psp,
            tc.tile_pool(name="pst", bufs=2, space="PSUM") as pstp,
            tc.tile_pool(name="pstf", bufs=2, space="PSUM") as pstfp,
            tc.tile_pool(name="dram", bufs=1, space="DRAM") as dramp,
        ):
            # ---- resident constants
            dinvb = constp.tile([128, NC], f16, tag="dinvb")
            dinvt = constp.tile([128, NC // 128], fp32, tag="dinvt")
            w0 = constp.tile([128, 256], f16, tag="w0")
            w1 = constp.tile([128, 512], f16, tag="w1")
            w2 = constp.tile([128, 256], f16, tag="w2")
            bnp = constp.tile([128, 10], fp32, tag="bnp")
            identb = constp.tile([128, 128], f16, tag="identb")
            identf = constp.tile([128, 128], fp32, tag="identf")
            for t, d in [(dinvb, dinvb_d), (dinvt, dinvt_d),
                         (w0, w0_d), (w1, w1_d),
                         (w2, w2_d), (bnp, bnp_d), (identb, identb_d),
                         (identf, identf_d)]:
                nc.sync.dma_start(t[:], d[:])

            tbl = dramp.tile([TR, DH], f16, tag="tbl", addr_space="Shared")
            tbl2 = dramp.tile([TR, DOUT], f16, tag="tbl2",
                              addr_space="Shared")
            agsrc = dramp.tile([NC, DH], f16, tag="agsrc")
            agsrc2 = dramp.tile([NC, DOUT], f16, tag="agsrc2")

            def gather_reduce(table, elem, blocks):
                """A+B gather phases -> (outA f16, outB f16) [128,2,*].

                elem: f16 elements per table row (128 or 256); a single
                gather call fetches the whole row (one descriptor/edge)."""
                outA = redp.tile([128, 2, NC], f16, tag="outA")
                outB = redp.tile([128, 2, NC], f16, tag="outB")
                with nc.allow_low_precision(reason="DVE accumulates fp32"):
                    for phase in ("A", "B"):
                        if phase == "A":
                            idxd, chunks, outX = \
                                idxa_d, sched["chunksA"], outA
                            view = table[0:min(TR, IMAX), :]
                        else:
                            idxd, chunks, outX = \
                                idxb_d, sched["chunksB"], outB
                            view = table[baseB:TR, :]
                        for k, chk in enumerate(chunks):
                            gbs = [gatp.tile([128, 2, CHA], f16,
                                             tag=f"gs{sx}", name=f"gs{sx}")
                                   for sx in range(SUB)]
                            idxt = gixp.tile([128, CHB // 16], i16,
                                             tag="idxt")
                            nc.sync.dma_start(
                                idxt[:], idxd[:, k * (CHB // 16):
                                              (k + 1) * (CHB // 16)])
                            for sx in range(SUB):
                                nc.gpsimd.dma_gather(
                                    out_ap=gbs[sx][:, :blocks, :],
                                    in_ap=view,
                                    idxs_ap=idxt[:, sx * (CHA // 16):
                                                 (sx + 1) * (CHA // 16)],
                                    num_idxs=CHA,
                                    num_idxs_reg=CHA,
                                    elem_size=elem,
                                    transpose=True,
                                    queue_num=sx % 4,
                                )
                            for (sub, off, ocol, n, g) in chk:
                                for j in range(blocks):
                                    nc.vector.tensor_reduce(
                                        outX[:, j, ocol:ocol + n],
                                        gbs[sub][:, j, off:off + n * g]
                                        .rearrange("p (n g) -> p n g", g=g),
                                        axis=mybir.AxisListType.X,
                                        op=OP.add,
                                    )
                return outA, outB

            def merge(outA, outB, blocks):
                """A+B add + dinv[dst] scale -> aggT f16 [128,2,NC]."""
                aggT = aggp.tile([128, 2, NC], f16, tag="aggbuf")
                for j in range(blocks):
                    nc.vector.tensor_tensor(aggT[:, j, :], outA[:, j, :],
                                            outB[:, j, :], OP.add)
                    nc.vector.tensor_tensor(aggT[:, j, :], aggT[:, j, :],
                                            dinvb[:], OP.mult)
                return aggT

            def bn_consts(st, blocks_out, bn_off, layer):
                """AllReduce stats -> per-feature scale A / bias B tiles."""
                stin = dramp.tile([128, 4], fp32, tag=f"stin{layer}")
                stout = dramp.tile([128, 4], fp32, tag=f"stout{layer}",
                                   addr_space="Shared")
                nc.gpsimd.dma_start(stin[:], st[:])
                nc.gpsimd.collective_compute(
                    "AllReduce", OP.add,
                    replica_groups=[list(range(NCORES))],
                    ins=[stin.opt()], outs=[stout.opt()],
                )
                stg = smallp.tile([128, 4], fp32, tag="stg")
                nc.sync.dma_start(stg[:], stout[:])
                b = blocks_out
                mu = smallp.tile([128, 2], fp32, tag="mu")
                va = smallp.tile([128, 2], fp32, tag="va")
                Ab = smallp.tile([128, 2], fp32, tag="Ab")
                Bb = smallp.tile([128, 2], fp32, tag="Bb")
                musq = smallp.tile([128, 2], fp32, tag="musq")
                rstd = smallp.tile([128, 2], fp32, tag="rstd")
                nc.vector.tensor_scalar(mu[:, :b], stg[:, 0:b], 1.0 / N, None,
                                        op0=OP.mult)
                nc.vector.tensor_scalar(va[:, :b], stg[:, 2:2 + b], 1.0 / N,
                                        None, op0=OP.mult)
                nc.vector.tensor_tensor(musq[:, :b], mu[:, :b], mu[:, :b],
                                        OP.mult)
                nc.vector.tensor_tensor(va[:, :b], va[:, :b], musq[:, :b],
                                        OP.subtract)
                sqv = smallp.tile([128, 2], fp32, tag="sqv")
                nc.vector.tensor_scalar(sqv[:, :b], va[:, :b], EPS, None,
                                        op0=OP.add)
                nc.scalar.activation(sqv[:, :b], sqv[:, :b], AF.Sqrt)
                nc.vector.reciprocal(rstd[:, :b], sqv[:, :b])
                gsl = bnp[:, bn_off:bn_off + b]
                bsl = bnp[:, bn_off + b:bn_off + 2 * b]
                nc.vector.tensor_tensor(Ab[:, :b], rstd[:, :b], gsl, OP.mult)
                nc.vector.tensor_tensor(Bb[:, :b], mu[:, :b], Ab[:, :b],
                                        OP.mult)
                nc.vector.tensor_tensor(Bb[:, :b], bsl, Bb[:, :b],
                                        OP.subtract)
                return Ab, Bb

            def conv_bn(aggT, wt, KT, bn_off, lrelu, layer):
                """matmul (out 2 blocks of 128) + BN(+lrelu) -> h f16."""
                cv = aggp.tile([128, 2, NC], f16, tag="aggbuf")
                ssum = smallp.tile([128, 2, NCH], fp32, tag="ssum")
                sqsum = smallp.tile([128, 2, NCH], fp32, tag="sqsum")
                for j in range(2):
                    for t in range(NCH):
                        ps = psp.tile([128, 512], fp32, tag="cps")
                        sl = slice(t * 512, (t + 1) * 512)
                        for kt in range(KT):
                            lhsT = wt[:, kt * 256 + j * 128:
                                      kt * 256 + (j + 1) * 128]
                            nc.tensor.matmul(ps[:], lhsT, aggT[:, kt, sl],
                                             start=(kt == 0),
                                             stop=(kt == KT - 1))
                        sq = smallp.tile([128, 512], f16, tag="sqd")
                        nc.scalar.activation(cv[:, j, sl], ps[:], AF.Copy,
                                             accum_out=ssum[:, j, t:t + 1])
                        nc.scalar.activation(sq[:], ps[:], AF.Square,
                                             accum_out=sqsum[:, j, t:t + 1])
                st = smallp.tile([128, 4], fp32, tag="stl")
                for j in range(2):
                    nc.vector.tensor_reduce(st[:, j:j + 1], ssum[:, j, :],
                                            axis=mybir.AxisListType.X,
                                            op=OP.add)
                    nc.vector.tensor_reduce(st[:, 2 + j:3 + j], sqsum[:, j, :],
                                            axis=mybir.AxisListType.X,
                                            op=OP.add)
                Ab, Bb = bn_consts(st, 2, bn_off, layer)
                h = aggp.tile([128, 2, NC], f16, tag="aggbuf")
                fn = AF.Lrelu if lrelu else AF.Identity
                for j in range(2):
                    nc.scalar.activation(h[:, j, :], cv[:, j, :], fn,
                                         bias=Bb[:, j:j + 1],
                                         scale=Ab[:, j:j + 1], alpha=SLOPE)
                return h, cv, st

            def write_rows(srcT, blocks, dst_dram, width, prescale):
                """transpose (+ optional dinv[src] scale post-transpose) +
                DMA rows."""
                for t in range(NC // 128):
                    row = rowp.tile([128, width], f16, tag="rowt")
                    for j in range(blocks):
                        pt = pstp.tile([128, 128], f16, tag="tps")
                        nc.tensor.transpose(
                            pt[:], srcT[:, j, t * 128:(t + 1) * 128],
                            identb[:])
                        if prescale:
                            nc.scalar.activation(
                                row[:, j * 128:(j + 1) * 128], pt[:],
                                AF.Copy, scale=dinvt[:, t:t + 1])
                        else:
                            nc.scalar.activation(
                                row[:, j * 128:(j + 1) * 128], pt[:],
                                AF.Copy)
                    nc.sync.dma_start(dst_dram[t * 128:(t + 1) * 128, :],
                                      row[:])

            # ================= layer 0 =================
            outA, outB = gather_reduce(xt_d, DIN, 1)
            if debug:
                nc.gpsimd.dma_start(dbg["dbg_outA0"][:], outA[:, 0, :])
                nc.gpsimd.dma_start(dbg["dbg_outB0"][:], outB[:, 0, :])
            aggT = merge(outA, outB, 1)
            if debug:
                nc.gpsimd.dma_start(dbg["dbg_agg0"][:], aggT[:, 0, :])
            h1, cv0, st0 = conv_bn(aggT, w0, 1, 0, True, 0)
            if debug:
                nc.gpsimd.dma_start(
                    dbg["dbg_h1"][:].rearrange("p (a b) -> p a b", a=2),
                    h1[:])
            write_rows(h1, 2, agsrc, DH, prescale=True)
            nc.gpsimd.collective_compute(
                "AllGather", OP.bypass,
                replica_groups=[list(range(NCORES))],
                ins=[agsrc.opt()], outs=[tbl[1:1 + NCORES * NC, :]],
            )

            # ================= layer 1 =================
            outA, outB = gather_reduce(tbl, DH, 2)
            if debug:
                nc.gpsimd.dma_start(
                    dbg["dbg_outA1"][:].rearrange("p (a b) -> p a b", a=2),
                    outA[:])
                nc.gpsimd.dma_start(
                    dbg["dbg_outB1"][:].rearrange("p (a b) -> p a b", a=2),
                    outB[:])
            aggT = merge(outA, outB, 2)
            if debug:
                nc.gpsimd.dma_start(
                    dbg["dbg_agg1"][:].rearrange("p (a b) -> p a b", a=2),
                    aggT[:])
            h2, _, _ = conv_bn(aggT, w1, 2, 4, True, 1)
            # transform-first for layer 2: T2 = W2 @ (dinv * h2)
            hs2 = aggp.tile([128, 2, NC], f16, tag="aggbuf")
            for j in range(2):
                nc.vector.tensor_tensor(hs2[:, j, :], h2[:, j, :], dinvb[:],
                                        OP.mult)
            t2 = aggp.tile([128, 2, NC], f16, tag="aggbuf")
            for t in range(NCH):
                ps = psp.tile([128, 512], fp32, tag="cps")
                sl = slice(t * 512, (t + 1) * 512)
                for kt in range(2):
                    nc.tensor.matmul(ps[:], w2[:, kt * 128:(kt + 1) * 128],
                                     hs2[:, kt, sl],
                                     start=(kt == 0), stop=(kt == 1))
                nc.scalar.activation(t2[:, 0, sl], ps[:], AF.Copy)
            write_rows(t2, 1, agsrc2, DOUT, prescale=False)
            nc.gpsimd.collective_compute(
                "AllGather", OP.bypass,
                replica_groups=[list(range(NCORES))],
                ins=[agsrc2.opt()], outs=[tbl2[1:1 + NCORES * NC, :]],
            )

            # ================= layer 2 =================
            outA, outB = gather_reduce(tbl2, DOUT, 1)
            aggT = merge(outA, outB, 1)
            # aggT IS the conv output (transform-first); BN only, no lrelu.
            ssum = smallp.tile([128, 2, NCH], fp32, tag="ssum")
            sqsum = smallp.tile([128, 2, NCH], fp32, tag="sqsum")
            for t in range(NCH):
                sl = slice(t * 512, (t + 1) * 512)
                sq = smallp.tile([128, 512], f16, tag="sqd")
                nc.scalar.activation(sq[:], aggT[:, 0, sl], AF.Square,
                                     accum_out=sqsum[:, 0, t:t + 1])
                nc.vector.tensor_reduce(ssum[:, 0, t:t + 1], aggT[:, 0, sl],
                                        axis=mybir.AxisListType.X, op=OP.add)
            st = smallp.tile([128, 4], fp32, tag="stl")
            nc.vector.tensor_reduce(st[:, 0:1], ssum[:, 0, :],
                                    axis=mybir.AxisListType.X, op=OP.add)
            nc.vector.tensor_reduce(st[:, 2:3], sqsum[:, 0, :],
                                    axis=mybir.AxisListType.X, op=OP.add)
            nc.vector.memset(st[:, 1:2], 0)
            nc.vector.memset(st[:, 3:4], 0)
            Ab, Bb = bn_consts(st, 1, 8, 2)
            for t in range(NC // 128):
                hf = smallp.tile([128, 128], fp32, tag="hfin")
                nc.scalar.activation(hf[:], aggT[:, 0, t * 128:(t + 1) * 128],
                                     AF.Identity,
                                     bias=Bb[:, 0:1], scale=Ab[:, 0:1])
                row = rowp.tile([128, DOUT], fp32, tag="rowtf")
                pt = pstfp.tile([128, 128], fp32, tag="tpsf")
                nc.tensor.transpose(pt[:], hf[:], identf[:])
                nc.vector.tensor_copy(row[:], pt[:])
                nc.sync.dma_start(out_d[t * 128:(t + 1) * 128, :], row[:])

    nc.compile()
    return nc


# ----------------------------------------------------------------------------
# entry point
# ----------------------------------------------------------------------------

def _make_inmaps(sched, data, W0, W1, W2, g0, be0, g1, be1, g2, be2):
    w0 = np.ascontiguousarray(W0.T.astype(F16))
    w1 = np.ascontiguousarray(
        W1.T.reshape(2, 128, 256).transpose(1, 0, 2).reshape(128, 512)
        .astype(F16))
    w2 = np.ascontiguousarray(
        W2.T.reshape(2, 128, 128).transpose(1, 0, 2).reshape(128, 256)
        .astype(F16))
    bnp = np.zeros((128, 10), np.float32)
    bnp[:, 0:2] = g0.reshape(2, 128).T
    bnp[:, 2:4] = be0.reshape(2, 128).T
    bnp[:, 4:6] = g1.reshape(2, 128).T
    bnp[:, 6:8] = be1.reshape(2, 128).T
    bnp[:, 8] = g2
    bnp[:, 9] = be2
    identb = np.eye(128, dtype=F16)
    identf = np.eye(128, dtype=np.float32)
    maps = []
    for c in range(NCORES):
        maps.append({
            "xt": data["xt"], "idxa": data["idxA"][c],
            "idxb": data["idxB"][c],
            "dinvb": data["dinvb"][c], "dinvt": data["dinvt"][c],
            "w0": w0, "w1": w1, "w2": w2, "bnp": bnp,
            "identb": identb, "identf": identf,
        })
    return maps


_CACHE = {}


def kernel(x, edge_index, W0, b0, g0, be0, W1, b1, g1, be1, W2, b2, g2, be2,
           _trace=False, _tmpdir=None, _debug=False):
    x = np.asarray(x, np.float32)
    edge_index = np.asarray(edge_index, np.int32)
    args = [np.asarray(a, np.float32)
            for a in (W0, b0, g0, be0, W1, b1, g1, be1, W2, b2, g2, be2)]
    (W0, b0, g0, be0, W1, b1, g1, be1, W2, b2, g2, be2) = args
    # conv bias cancels exactly in training-mode BatchNorm -> ignored.

    key = (edge_index.tobytes()[:256], int(edge_index.sum()), bool(_debug))
    if key not in _CACHE:
        sched, data = preprocess(edge_index, x)
        nc_obj = build_nc(sched, debug=_debug)
        _CACHE[key] = (sched, nc_obj)
    else:
        sched, nc_obj = _CACHE[key]
        _, data = preprocess(edge_index, x)

    in_maps = _make_inmaps(sched, data, W0, W1, W2, g0, be0, g1, be1, g2, be2)
    res = run_bass_kernel_spmd(nc_obj, in_maps, core_ids=list(range(NCORES)),
                               trace=_trace, tmpdir=_tmpdir)

    out = np.zeros((N, DOUT), np.float32)
    for c in range(NCORES):
        o = np.asarray(res.results[c]["out"])
        sn = sched["slot_node"][c]
        real = sn >= 0
        out[sn[real]] = o[real]
    kernel._last_result = res
    kernel._last_sched = sched
    return out


# revision 20
# speedup vs baseline: 2.1969x; 1.0162x over previous
"""GCN 3-layer forward on 8 Trainium2 NeuronCores (Bass/Tile).

Self-contained: hardcodes the problem shapes from the spec.
kernel(**inputs) -> np.ndarray [50000, 128] float32.

Layout: feature-major ("transposed") on chip — features on partitions,
nodes along the free dim.  Nodes are degree-sorted and round-robin
assigned to cores; per-core slots sorted by (A-half degree, B-half
degree) so the segmented reduce is a short list of constant-degree runs
shared by all cores (histograms padded to a common shape).  Message
gather uses dma_gather(transpose=True) from f16 tables in DRAM; the
int16 index limit is handled by an A/B split of the table (cores 0-3
via view [0,32768), cores 4-7 via [TR-32768,TR)).  Layer 1 gathers full
512B rows (elem_size=256 -> [128,2,CHA] out) so each edge costs ONE
SWDGE descriptor on every layer.  Gather streams are packed so no
segment crosses a 896-index sub-call boundary.  The B-phase output is
permuted back to slot order with ap_gather over fp32-paired f16
columns.  Symmetric normalization is factorized: table rows pre-scaled
by dinv[src], aggregates post-scaled by dinv[dst].  Conv bias is
dropped (cancels exactly in training-mode BatchNorm).  BN+LeakyReLU is
one ScalarE activation; BN stats ride accum_out + a tiny AllReduce;
tables are shared with AllGather.
"""
import sys

sys.path.insert(0, "/opt/trn_rl_repo")

import numpy as np
import ml_dtypes

import concourse.bacc as bacc
import concourse.mybir as mybir
import concourse.tile as tile
from concourse.bass_utils import run_bass_kernel_spmd

F16 = np.float16

N, E, DIN, DH, DOUT = 50000, 800000, 128, 256, 128
EPS = 1e-5
SLOPE = 0.01
NCORES = 8
ACORES = 4          # cores 0..3 form the "A" half of the table
CHA = 896           # idxs per gather call
SUB = 4             # gather calls per chunk (one idx-DMA granularity)
CHB = CHA * SUB     # edges per chunk
IMAX = 32768        # int16 index window


# ----------------------------------------------------------------------------
# host-side schedule construction
# ----------------------------------------------------------------------------

def _wrap_idx(arr):
    """1-D int array -> [128, len/16] int16 wrapped+replicated layout."""
    L = len(arr)
    assert L % 16 == 0
    a = np.asarray(arr, np.int16).reshape(L // 16, 16).T  # [16, L/16]
    return np.tile(a, (8, 1)).copy()  # [128, L/16]


def _pack_phase(gvec):
    """Pack even-degree columns into a CHA-subdivided stream.

    gvec: per-column even degrees (bucket-sorted ascending).  Columns are
    packed left-to-right; padding keeps any column's span inside one
    CHA-sized sub-call, and runs split at even column counts (4B output
    alignment).  Returns (runs, colpos, E): runs = [(pos, jcol, n, g)],
    colpos[j] = stream position of column j's first edge, E = padded
    stream length (multiple of CHB).
    """
    M = len(gvec)
    colpos = np.zeros(M, np.int64)
    runs = []
    pos = 0
    j = 0
    while j < M:
        g = int(gvec[j])
        if g == 0:
            j += 1
            continue
        assert 2 * g <= CHA, f"degree {g} too large for sub-call"
        k = j
        while k < M and gvec[k] == g:
            k += 1
        while j < k:
            rem = CHA - (pos % CHA)
            ncols_fit = (rem // g) & ~1
            if ncols_fit < 2:
                pos += rem  # pad to next sub boundary
                continue
            take = min(k - j, ncols_fit)
            runs.append((int(pos), int(j), int(take), int(g)))
            colpos[j:j + take] = pos + np.arange(take) * g
            pos += take * g
            j += take
    Epad = -(-pos // CHB) * CHB
    return runs, colpos, Epad


def _sched_from_runs(runs, Epad):
    """Group runs by chunk; a run never crosses a sub boundary."""
    nchunks = Epad // CHB
    by_chunk = [[] for _ in range(nchunks)]
    for (pos, jcol, n, g) in runs:
        chunk, rem = divmod(pos, CHB)
        sub, off = divmod(rem, CHA)
        assert off + n * g <= CHA
        by_chunk[chunk].append((sub, off, jcol, n, g))
    return by_chunk


def _balance_sides(src, dst, rounds=60, seed=0):
    """Side (+1=A, -1=B) per node so each dst's in-edges split ~evenly
    (gA = ceil(g/2) target) and |A| == N/2.  Greedy local search."""
    g = np.bincount(dst, minlength=N)
    target = (g & 1).astype(np.int64)
    k_out = np.bincount(src, minlength=N).astype(np.int64)
    o = np.argsort(g, kind="stable")
    s = np.empty(N, np.int64)
    s[o] = np.where(np.arange(N) % 2 == 0, 1, -1)
    rng = np.random.default_rng(seed)
    best_err, best_s = None, s.copy()
    for _ in range(rounds):
        dev = np.bincount(dst, weights=s[src].astype(np.float64),
                          minlength=N).astype(np.int64) - target
        err = int(np.abs(dev).sum())
        if best_err is None or err < best_err:
            best_err, best_s = err, s.copy()
        if err == 0:
            break
        W = np.bincount(src, weights=dev[dst].astype(np.float64),
                        minlength=N).astype(np.int64)
        gain = s * W - k_out
        candA = np.flatnonzero((s > 0) & (gain > 0))
        candB = np.flatnonzero((s < 0) & (gain > 0))
        m = min(len(candA), len(candB))
        if m == 0:
            break
        m = max(1, int(m * 0.35))
        s[rng.choice(candA, m, replace=False)] *= -1
        s[rng.choice(candB, m, replace=False)] *= -1
    return best_s


def preprocess(edge_index, x):
    src0 = np.asarray(edge_index[0], np.int64)
    dst0 = np.asarray(edge_index[1], np.int64)
    loop = np.arange(N, dtype=np.int64)
    src = np.concatenate([src0, loop])
    dst = np.concatenate([dst0, loop])

    deg = np.bincount(dst, minlength=N)
    dinv = (1.0 / np.sqrt(deg.astype(np.float64))).astype(np.float32)

    sides = _balance_sides(src, dst)
    a_mask = sides[src] > 0
    gA = np.bincount(dst[a_mask], minlength=N)
    gB = deg - gA
    gAp = gA + (gA & 1)   # even-padded phase degrees
    gBp = gB + (gB & 1)

    # core assignment: within each side, (gAp,gBp)-sorted round robin
    core_of = np.empty(N, np.int64)
    for sd, base in [(1, 0), (-1, 4)]:
        nodes = np.flatnonzero(sides == sd)
        o2 = nodes[np.lexsort((gBp[nodes], gAp[nodes]))]
        core_of[o2] = base + np.arange(len(o2)) % 4

    cnts = [int((core_of == c).sum()) for c in range(NCORES)]
    NC = -(-(2 + max(cnts)) // 512) * 512
    assert 4 * NC < IMAX

    # slots 0,1 reserved as zero rows; nodes at slots 2.. in lexsort order
    slot_node = []
    avec = np.zeros(NC, np.int64)
    bvec = np.zeros(NC, np.int64)
    for c in range(NCORES):
        nodes_c = np.flatnonzero(core_of == c)
        nodes_c = nodes_c[np.lexsort((gBp[nodes_c], gAp[nodes_c]))]
        sn = np.full(NC, -1, np.int64)
        sn[2:2 + len(nodes_c)] = nodes_c
        slot_node.append(sn)
        La = np.zeros(NC, np.int64)
        Lb = np.zeros(NC, np.int64)
        La[2:2 + len(nodes_c)] = gAp[nodes_c]
        Lb[2:2 + len(nodes_c)] = gBp[nodes_c]
        avec = np.maximum(avec, La)
        bvec = np.maximum(bvec, Lb)
    avec = np.maximum(avec, 2)   # no-memset: every slot reduces >= 2 entries
    bvec = np.maximum(bvec, 2)

    slot_of = np.full(N, -1, np.int64)
    for c in range(NCORES):
        real = slot_node[c] >= 0
        slot_of[slot_node[c][real]] = np.flatnonzero(real)

    TR = 2 + NCORES * NC
    baseB = max(0, TR - IMAX)
    row_of = 1 + core_of * NC + slot_of

    assert int(row_of[core_of < ACORES].max()) < IMAX
    assert int(row_of[core_of >= ACORES].min()) >= baseB

    zA = 1                      # core 0, slot 0 (always a zero row)
    zB = 1 + 7 * NC             # core 7, slot 0
    runsA, colposA, EA = _pack_phase(avec)
    runsB, colposB, EB = _pack_phase(bvec)
    chunksA = _sched_from_runs(runsA, EA)
    chunksB = _sched_from_runs(runsB, EB)

    idxA, idxB, dinvb_l, dinvt_l = [], [], [], []
    for c in range(NCORES):
        # --- A stream (default idx = zero row)
        sA = np.full(EA, zA, np.int64)
        m = (core_of[dst] == c) & a_mask
        es, cols = src[m], slot_of[dst[m]]
        o = np.argsort(cols, kind="stable")
        es, cols = es[o], cols[o]
        ranks = np.arange(len(cols)) - np.searchsorted(cols, cols)
        sA[colposA[cols] + ranks] = row_of[es]
        assert sA.max() < IMAX
        idxA.append(_wrap_idx(sA))

        # --- B stream (default = B zero row), direct slot columns
        sB = np.full(EB, zB - baseB, np.int64)
        m = (core_of[dst] == c) & (~a_mask)
        es, cols = src[m], slot_of[dst[m]]
        o = np.argsort(cols, kind="stable")
        es, cols = es[o], cols[o]
        ranks = np.arange(len(cols)) - np.searchsorted(cols, cols)
        sB[colposB[cols] + ranks] = row_of[es] - baseB
        assert sB.min() >= 0 and sB.max() < IMAX
        idxB.append(_wrap_idx(sB))

        dv = np.zeros(NC, np.float32)
        real = slot_node[c] >= 0
        dv[real] = dinv[slot_node[c][real]]
        dinvb_l.append(np.tile(dv[None, :].astype(F16), (128, 1)))
        dinvt_l.append(np.ascontiguousarray(
            dv.reshape(NC // 128, 128).T.astype(np.float32)))

    xt = np.zeros((TR, DIN), F16)
    xs = (np.asarray(x, np.float32) * dinv[:, None]).astype(F16)
    xt[row_of] = xs

    sched = {
        "NC": NC, "TR": TR, "baseB": baseB,
        "chunksA": chunksA, "chunksB": chunksB, "EA": EA, "EB": EB,
        "slot_node": slot_node,
    }
    data = {"xt": xt, "idxA": idxA, "idxB": idxB,
            "dinvb": dinvb_l, "dinvt": dinvt_l}
    return sched, data


# ----------------------------------------------------------------------------
# device kernel builder
# ----------------------------------------------------------------------------

def build_nc(sched, debug=False):
    NC, TR = sched["NC"], sched["TR"]
    baseB = sched["baseB"]
    EA, EB = sched["EA"], sched["EB"]
    fp32 = mybir.dt.float32
    f16 = mybir.dt.float16
    i16 = mybir.dt.int16
    AF = mybir.ActivationFunctionType
    OP = mybir.AluOpType
    NCH = NC // 512

    nc = bacc.Bacc("TRN2", target_bir_lowering=False, num_devices=NCORES,
                   num_swdge_queues=4, dynamic_dma_scratch_size=16384)

    xt_d = nc.dram_tensor("xt", [TR, DIN], f16, kind="ExternalInput")
    idxa_d = nc.dram_tensor("idxa", [128, EA // 16], i16, kind="ExternalInput")
    idxb_d = nc.dram_tensor("idxb", [128, EB // 16], i16, kind="ExternalInput")
    dinvb_d = nc.dram_tensor("dinvb", [128, NC], f16, kind="ExternalInput")
    dinvt_d = nc.dram_tensor("dinvt", [128, NC // 128], fp32,
                             kind="ExternalInput")
    w0_d = nc.dram_tensor("w0", [128, 256], f16, kind="ExternalInput")
    w1_d = nc.dram_tensor("w1", [128, 512], f16, kind="ExternalInput")
    w2_d = nc.dram_tensor("w2", [128, 256], f16, kind="ExternalInput")
    bnp_d = nc.dram_tensor("bnp", [128, 10], fp32, kind="ExternalInput")
    identb_d = nc.dram_tensor("identb", [128, 128], f16, kind="ExternalInput")
    identf_d = nc.dram_tensor("identf", [128, 128], fp32, kind="ExternalInput")
    out_d = nc.dram_tensor("out", [NC, DOUT], fp32, kind="ExternalOutput")
    dbg = {}
    if debug:
        for name, shape in [
            ("dbg_outA0", [128, NC]), ("dbg_outB0", [128, NC]),
            ("dbg_agg0", [128, NC]), ("dbg_h1", [128, 2 * NC]),
            ("dbg_agg1", [128, 2 * NC]), ("dbg_outA1", [128, 2 * NC]),
            ("dbg_outB1", [128, 2 * NC]),
        ]:
            dbg[name] = nc.dram_tensor(name, shape, fp32,
                                       kind="ExternalOutput")

    with tile.TileContext(nc) as tc:
        with (
            tc.tile_pool(name="const", bufs=1) as constp,
            tc.tile_pool(name="gat", bufs=3) as gatp,
            tc.tile_pool(name="gix", bufs=3) as gixp,
            tc.tile_pool(name="red", bufs=1) as redp,
            tc.tile_pool(name="agg", bufs=2) as aggp,
            tc.tile_pool(name="small", bufs=2) as smallp,
            tc.tile_pool(name="rowt", bufs=2) as rowp,
            tc.tile_pool(name="ps", bufs=3, space="PSUM") as psp,
            tc.tile_pool(name="pst", bufs=2, space="PSUM") as pstp,
            tc.tile_pool(name="pstf", bufs=2, space="PSUM") as pstfp,
            tc.tile_pool(name="dram", bufs=1, space="DRAM") as dramp,
        ):
            # ---- resident constants
            dinvb = constp.tile([128, NC], f16, tag="dinvb")
            dinvt = constp.tile([128, NC // 128], fp32, tag="dinvt")
            w0 = constp.tile([128, 256], f16, tag="w0")
            w1 = constp.tile([128, 512], f16, tag="w1")
            w2 = constp.tile([128, 256], f16, tag="w2")
            bnp = constp.tile([128, 10], fp32, tag="bnp")
            identb = constp.tile([128, 128], f16, tag="identb")
            identf = constp.tile([128, 128], fp32, tag="identf")
            for t, d in [(dinvb, dinvb_d), (dinvt, dinvt_d),
                         (w0, w0_d), (w1, w1_d),
                         (w2, w2_d), (bnp, bnp_d), (identb, identb_d),
                         (identf, identf_d)]:
                nc.sync.dma_start(t[:], d[:])

            tbl = dramp.tile([TR, DH], f16, tag="tbl", addr_space="Shared")
            tbl2 = dramp.tile([TR, DOUT], f16, tag="tbl2",
                              addr_space="Shared")
            agsrc = dramp.tile([NC, DH], f16, tag="agsrc")
            agsrc2 = dramp.tile([NC, DOUT], f16, tag="agsrc2")

            def gather_reduce(table, elem, blocks):
                """A+B gather phases -> (outA f16, outB f16) [128,2,*].

                elem: f16 elements per table row (128 or 256); a single
                gather call fetches the whole row (one descriptor/edge)."""
                outA = redp.tile([128, 2, NC], f16, tag="outA")
                outB = redp.tile([128, 2, NC], f16, tag="outB")
                with nc.allow_low_precision(reason="DVE accumulates fp32"):
                    for phase in ("A", "B"):
                        if phase == "A":
                            idxd, chunks, outX = \
                                idxa_d, sched["chunksA"], outA
                            view = table[0:min(TR, IMAX), :]
                        else:
                            idxd, chunks, outX = \
                                idxb_d, sched["chunksB"], outB
                            view = table[baseB:TR, :]
                        for k, chk in enumerate(chunks):
                            gbs = [gatp.tile([128, 2, CHA], f16,
                                             tag=f"gs{sx}", name=f"gs{sx}")
                                   for sx in range(SUB)]
                            idxt = gixp.tile([128, CHB // 16], i16,
                                             tag="idxt")
                            nc.sync.dma_start(
                                idxt[:], idxd[:, k * (CHB // 16):
                                              (k + 1) * (CHB // 16)])
                            for sx in range(SUB):
                                nc.gpsimd.dma_gather(
                                    out_ap=gbs[sx][:, :blocks, :],
                                    in_ap=view,
                                    idxs_ap=idxt[:, sx * (CHA // 16):
                                                 (sx + 1) * (CHA // 16)],
                                    num_idxs=CHA,
                                    num_idxs_reg=CHA,
                                    elem_size=elem,
                                    transpose=True,
                                    queue_num=sx % 4,
                                )
                            for (sub, off, ocol, n, g) in chk:
                                for j in range(blocks):
                                    nc.vector.tensor_reduce(
                                        outX[:, j, ocol:ocol + n],
                                        gbs[sub][:, j, off:off + n * g]
                                        .rearrange("p (n g) -> p n g", g=g),
                                        axis=mybir.AxisListType.X,
                                        op=OP.add,
                                    )
                return outA, outB

            def merge(outA, outB, blocks):
                """A+B add + dinv[dst] scale -> aggT f16 [128,2,NC]."""
                aggT = aggp.tile([128, 2, NC], f16, tag="aggbuf")
                for j in range(blocks):
                    nc.vector.tensor_tensor(aggT[:, j, :], outA[:, j, :],
                                            outB[:, j, :], OP.add)
                    nc.vector.tensor_tensor(aggT[:, j, :], aggT[:, j, :],
                                            dinvb[:], OP.mult)
                return aggT

            def bn_consts(st, blocks_out, bn_off, layer):
                """AllReduce stats -> per-feature scale A / bias B tiles."""
                stin = dramp.tile([128, 4], fp32, tag=f"stin{layer}")
                stout = dramp.tile([128, 4], fp32, tag=f"stout{layer}",
                                   addr_space="Shared")
                nc.gpsimd.dma_start(stin[:], st[:])
                nc.gpsimd.collective_compute(
                    "AllReduce", OP.add,
                    replica_groups=[list(range(NCORES))],
                    ins=[stin.opt()], outs=[stout.opt()],
                )
                stg = smallp.tile([128, 4], fp32, tag="stg")
                nc.sync.dma_start(stg[:], stout[:])
                b = blocks_out
                mu = smallp.tile([128, 2], fp32, tag="mu")
                va = smallp.tile([128, 2], fp32, tag="va")
                Ab = smallp.tile([128, 2], fp32, tag="Ab")
                Bb = smallp.tile([128, 2], fp32, tag="Bb")
                musq = smallp.tile([128, 2], fp32, tag="musq")
                rstd = smallp.tile([128, 2], fp32, tag="rstd")
                nc.vector.tensor_scalar(mu[:, :b], stg[:, 0:b], 1.0 / N, None,
                                        op0=OP.mult)
                nc.vector.tensor_scalar(va[:, :b], stg[:, 2:2 + b], 1.0 / N,
                                        None, op0=OP.mult)
                nc.vector.tensor_tensor(musq[:, :b], mu[:, :b], mu[:, :b],
                                        OP.mult)
                nc.vector.tensor_tensor(va[:, :b], va[:, :b], musq[:, :b],
                                        OP.subtract)
                sqv = smallp.tile([128, 2], fp32, tag="sqv")
                nc.vector.tensor_scalar(sqv[:, :b], va[:, :b], EPS, None,
                                        op0=OP.add)
                nc.scalar.activation(sqv[:, :b], sqv[:, :b], AF.Sqrt)
                nc.vector.reciprocal(rstd[:, :b], sqv[:, :b])
                gsl = bnp[:, bn_off:bn_off + b]
                bsl = bnp[:, bn_off + b:bn_off + 2 * b]
                nc.vector.tensor_tensor(Ab[:, :b], rstd[:, :b], gsl, OP.mult)
                nc.vector.tensor_tensor(Bb[:, :b], mu[:, :b], Ab[:, :b],
                                        OP.mult)
                nc.vector.tensor_tensor(Bb[:, :b], bsl, Bb[:, :b],
                                        OP.subtract)
                return Ab, Bb

            def conv_bn(aggT, wt, KT, bn_off, lrelu, layer):
                """matmul (out 2 blocks of 128) + BN(+lrelu) -> h f16."""
                cv = aggp.tile([128, 2, NC], f16, tag="aggbuf")
                ssum = smallp.tile([128, 2, NCH], fp32, tag="ssum")
                sqsum = smallp.tile([128, 2, NCH], fp32, tag="sqsum")
                for j in range(2):
                    for t in range(NCH):
                        ps = psp.tile([128, 512], fp32, tag="cps")
                        sl = slice(t * 512, (t + 1) * 512)
                        for kt in range(KT):
                            lhsT = wt[:, kt * 256 + j * 128:
                                      kt * 256 + (j + 1) * 128]
                            nc.tensor.matmul(ps[:], lhsT, aggT[:, kt, sl],
                                             start=(kt == 0),
                                             stop=(kt == KT - 1))
                        sq = smallp.tile([128, 512], f16, tag="sqd")
                        nc.scalar.activation(cv[:, j, sl], ps[:], AF.Copy,
                                             accum_out=ssum[:, j, t:t + 1])
                        nc.scalar.activation(sq[:], ps[:], AF.Square,
                                             accum_out=sqsum[:, j, t:t + 1])
                st = smallp.tile([128, 4], fp32, tag="stl")
                for j in range(2):
                    nc.vector.tensor_reduce(st[:, j:j + 1], ssum[:, j, :],
                                            axis=mybir.AxisListType.X,
                                            op=OP.add)
                    nc.vector.tensor_reduce(st[:, 2 + j:3 + j], sqsum[:, j, :],
                                            axis=mybir.AxisListType.X,
                                            op=OP.add)
                Ab, Bb = bn_consts(st, 2, bn_off, layer)
                h = aggp.tile([128, 2, NC], f16, tag="aggbuf")
                fn = AF.Lrelu if lrelu else AF.Identity
                for j in range(2):
                    nc.scalar.activation(h[:, j, :], cv[:, j, :], fn,
                                         bias=Bb[:, j:j + 1],
                                         scale=Ab[:, j:j + 1], alpha=SLOPE)
                return h, cv, st

            def write_rows(srcT, blocks, dst_dram, width, prescale):
                """transpose (+ optional dinv[src] scale post-transpose) +
                DMA rows."""
                for t in range(NC // 128):
                    row = rowp.tile([128, width], f16, tag="rowt")
                    for j in range(blocks):
                        pt = pstp.tile([128, 128], f16, tag="tps")
                        nc.tensor.transpose(
                            pt[:], srcT[:, j, t * 128:(t + 1) * 128],
                            identb[:])
                        if prescale:
                            nc.scalar.activation(
                                row[:, j * 128:(j + 1) * 128], pt[:],
                                AF.Copy, scale=dinvt[:, t:t + 1])
                        else:
                            nc.scalar.activation(
                                row[:, j * 128:(j + 1) * 128], pt[:],
                                AF.Copy)
                    nc.sync.dma_start(dst_dram[t * 128:(t + 1) * 128, :],
                                      row[:])

            # ================= layer 0 =================
            outA, outB = gather_reduce(xt_d, DIN, 1)
            if debug:
                nc.gpsimd.dma_start(dbg["dbg_outA0"][:], outA[:, 0, :])
                nc.gpsimd.dma_start(dbg["dbg_outB0"][:], outB[:, 0, :])
            aggT = merge(outA, outB, 1)
            if debug:
                nc.gpsimd.dma_start(dbg["dbg_agg0"][:], aggT[:, 0, :])
            h1, cv0, st0 = conv_bn(aggT, w0, 1, 0, True, 0)
            if debug:
                nc.gpsimd.dma_start(
                    dbg["dbg_h1"][:].rearrange("p (a b) -> p a b", a=2),
                    h1[:])
            write_rows(h1, 2, agsrc, DH, prescale=True)
            nc.gpsimd.collective_compute(
                "AllGather", OP.bypass,
                replica_groups=[list(range(NCORES))],
                ins=[agsrc.opt()], outs=[tbl[1:1 + NCORES * NC, :]],
            )

            # ================= layer 1 =================
            outA, outB = gather_reduce(tbl, DH, 2)
            if debug:
                nc.gpsimd.dma_start(
                    dbg["dbg_outA1"][:].rearrange("p (a b) -> p a b", a=2),
                    outA[:])
                nc.gpsimd.dma_start(
                    dbg["dbg_outB1"][:].rearrange("p (a b) -> p a b", a=2),
                    outB[:])
            aggT = merge(outA, outB, 2)
            if debug:
                nc.gpsimd.dma_start(
                    dbg["dbg_agg1"][:].rearrange("p (a b) -> p a b", a=2),
                    aggT[:])
            h2, _, _ = conv_bn(aggT, w1, 2, 4, True, 1)
            # transform-first for layer 2: T2 = W2 @ (dinv * h2)
            hs2 = aggp.tile([128, 2, NC], f16, tag="aggbuf")
            for j in range(2):
                nc.vector.tensor_tensor(hs2[:, j, :], h2[:, j, :], dinvb[:],
                                        OP.mult)
            t2 = aggp.tile([128, 2, NC], f16, tag="aggbuf")
            for t in range(NCH):
                ps = psp.tile([128, 512], fp32, tag="cps")
                sl = slice(t * 512, (t + 1) * 512)
                for kt in range(2):
                    nc.tensor.matmul(ps[:], w2[:, kt * 128:(kt + 1) * 128],
                                     hs2[:, kt, sl],
                                     start=(kt == 0), stop=(kt == 1))
                nc.scalar.activation(t2[:, 0, sl], ps[:], AF.Copy)
            write_rows(t2, 1, agsrc2, DOUT, prescale=False)
            nc.gpsimd.collective_compute(
                "AllGather", OP.bypass,
                replica_groups=[list(range(NCORES))],
                ins=[agsrc2.opt()], outs=[tbl2[1:1 + NCORES * NC, :]],
            )

            # ================= layer 2 =================
            outA, outB = gather_reduce(tbl2, DOUT, 1)
            aggT = merge(outA, outB, 1)
            # aggT IS the conv output (transform-first); BN only, no lrelu.
            ssum = smallp.tile([128, 2, NCH], fp32, tag="ssum")
            sqsum = smallp.tile([128, 2, NCH], fp32, tag="sqsum")
            for t in range(NCH):
                sl = slice(t * 512, (t + 1) * 512)
                sq = smallp.tile([128, 512], f16, tag="sqd")
                nc.scalar.activation(sq[:], aggT[:, 0, sl], AF.Square,
                                     accum_out=sqsum[:, 0, t:t + 1])
                nc.vector.tensor_reduce(ssum[:, 0, t:t + 1], aggT[:, 0, sl],
                                        axis=mybir.AxisListType.X, op=OP.add)
            st = smallp.tile([128, 4], fp32, tag="stl")
            nc.vector.tensor_reduce(st[:, 0:1], ssum[:, 0, :],
                                    axis=mybir.AxisListType.X, op=OP.add)
            nc.vector.tensor_reduce(st[:, 2:3], sqsum[:, 0, :],
                                    axis=mybir.AxisListType.X, op=OP.add)
            nc.vector.memset(st[:, 1:2], 0)
            nc.vector.memset(st[:, 3:4], 0)
            Ab, Bb = bn_consts(st, 1, 8, 2)
            for t in range(NC // 128):
                hf = smallp.tile([128, 128], fp32, tag="hfin")
                nc.scalar.activation(hf[:], aggT[:, 0, t * 128:(t + 1) * 128],
                                     AF.Identity,
                                     bias=Bb[:, 0:1], scale=Ab[:, 0:1])
                row = rowp.tile([128, DOUT], fp32, tag="rowtf")
                pt = pstfp.tile([128, 128], fp32, tag="tpsf")
                nc.tensor.transpose(pt[:], hf[:], identf[:])
                nc.vector.tensor_copy(row[:], pt[:])
                nc.sync.dma_start(out_d[t * 128:(t + 1) * 128, :], row[:])

    nc.compile()
    return nc


# ----------------------------------------------------------------------------
# entry point
# ----------------------------------------------------------------------------

def _make_inmaps(sched, data, W0, W1, W2, g0, be0, g1, be1, g2, be2):
    w0 = np.ascontiguousarray(W0.T.astype(F16))
    w1 = np.ascontiguousarray(
        W1.T.reshape(2, 128, 256).transpose(1, 0, 2).reshape(128, 512)
        .astype(F16))
    w2 = np.ascontiguousarray(
        W2.T.reshape(2, 128, 128).transpose(1, 0, 2).reshape(128, 256)
        .astype(F16))
    bnp = np.zeros((128, 10), np.float32)
    bnp[:, 0:2] = g0.reshape(2, 128).T
    bnp[:, 2:4] = be0.reshape(2, 128).T
    bnp[:, 4:6] = g1.reshape(2, 128).T
    bnp[:, 6:8] = be1.reshape(2, 128).T
    bnp[:, 8] = g2
    bnp[:, 9] = be2
    identb = np.eye(128, dtype=F16)
    identf = np.eye(128, dtype=np.float32)
    maps = []
    for c in range(NCORES):
        maps.append({
            "xt": data["xt"], "idxa": data["idxA"][c],
            "idxb": data["idxB"][c],
            "dinvb": data["dinvb"][c], "dinvt": data["dinvt"][c],
            "w0": w0, "w1": w1, "w2": w2, "bnp": bnp,
            "identb": identb, "identf": identf,
        })
    return maps


_CACHE = {}


def kernel(x, edge_index, W0, b0, g0, be0, W1, b1, g1, be1, W2, b2, g2, be2,
           _trace=False, _tmpdir=None, _debug=False):
    x = np.asarray(x, np.float32)
    edge_index = np.asarray(edge_index, np.int32)
    args = [np.asarray(a, np.float32)
            for a in (W0, b0, g0, be0, W1, b1, g1, be1, W2, b2, g2, be2)]
    (W0, b0, g0, be0, W1, b1, g1, be1, W2, b2, g2, be2) = args
    # conv bias cancels exactly in training-mode BatchNorm -> ignored.

    key = (edge_index.tobytes()[:256], int(edge_index.sum()), bool(_debug))
    if key not in _CACHE:
        sched, data = preprocess(edge_index, x)
        nc_obj = build_nc(sched, debug=_debug)
        _CACHE[key] = (sched, nc_obj)
    else:
        sched, nc_obj = _CACHE[key]
        _, data = preprocess(edge_index, x)

    in_maps = _make_inmaps(sched, data, W0, W1, W2, g0, be0, g1, be1, g2, be2)
    res = run_bass_kernel_spmd(nc_obj, in_maps, core_ids=list(range(NCORES)),
                               trace=_trace, tmpdir=_tmpdir)

    out = np.zeros((N, DOUT), np.float32)
    for c in range(NCORES):
        o = np.asarray(res.results[c]["out"])
        sn = sched["slot_node"][c]
        real = sn >= 0
        out[sn[real]] = o[real]
    kernel._last_result = res
    kernel._last_sched = sched
    return out


# revision 23
# speedup vs baseline: 2.2506x; 1.0244x over previous
"""GCN 3-layer forward on 8 Trainium2 NeuronCores (Bass/Tile).

Self-contained: hardcodes the problem shapes from the spec.
kernel(**inputs) -> np.ndarray [50000, 128] float32.

Layout: feature-major ("transposed") on chip — features on partitions,
nodes along the free dim.  Nodes are degree-sorted and round-robin
assigned to cores; per-core slots sorted by (A-half degree, B-half
degree) so the segmented reduce is a short list of constant-degree runs
shared by all cores (histograms padded to a common shape).  Message
gather uses dma_gather(transpose=True) from f16 tables in DRAM; the
int16 index limit is handled by an A/B split of the table (cores 0-3
via view [0,32768), cores 4-7 via [TR-32768,TR)).  Layer 1 gathers full
512B rows (elem_size=256 -> [128,2,CHA] out) so each edge costs ONE
SWDGE descriptor on every layer.  Gather streams are packed so no
segment crosses a 896-index sub-call boundary.  The B-phase output is
permuted back to slot order with ap_gather over fp32-paired f16
columns.  Symmetric normalization is factorized: table rows pre-scaled
by dinv[src], aggregates post-scaled by dinv[dst].  Conv bias is
dropped (cancels exactly in training-mode BatchNorm).  BN+LeakyReLU is
one ScalarE activation; BN stats ride accum_out + a tiny AllReduce;
tables are shared with AllGather.
"""
import sys

sys.path.insert(0, "/opt/trn_rl_repo")

import numpy as np
import ml_dtypes

import concourse.bacc as bacc
import concourse.mybir as mybir
import concourse.tile as tile
from concourse.bass_utils import run_bass_kernel_spmd

F16 = np.float16

N, E, DIN, DH, DOUT = 50000, 800000, 128, 256, 128
EPS = 1e-5
SLOPE = 0.01
NCORES = 8
ACORES = 4          # cores 0..3 form the "A" half of the table
CHA = 896           # idxs per gather call
SUB = 4             # gather calls per chunk (one idx-DMA granularity)
CHB = CHA * SUB     # edges per chunk
IMAX = 32768        # int16 index window


# ----------------------------------------------------------------------------
# host-side schedule construction
# ----------------------------------------------------------------------------

def _wrap_idx(arr):
    """1-D int array -> [128, len/16] int16 wrapped+replicated layout."""
    L = len(arr)
    assert L % 16 == 0
    a = np.asarray(arr, np.int16).reshape(L // 16, 16).T  # [16, L/16]
    return np.tile(a, (8, 1)).copy()  # [128, L/16]


def _pack_phase(gvec):
    """Pack even-degree columns into a CHA-subdivided stream.

    gvec: per-column even degrees (bucket-sorted ascending).  Columns are
    packed left-to-right; padding keeps any column's span inside one
    CHA-sized sub-call, and runs split at even column counts (4B output
    alignment).  Returns (runs, colpos, E): runs = [(pos, jcol, n, g)],
    colpos[j] = stream position of column j's first edge, E = padded
    stream length (multiple of CHB).
    """
    M = len(gvec)
    colpos = np.zeros(M, np.int64)
    runs = []
    pos = 0
    j = 0
    while j < M:
        g = int(gvec[j])
        if g == 0:
            j += 1
            continue
        assert 2 * g <= CHA, f"degree {g} too large for sub-call"
        k = j
        while k < M and gvec[k] == g:
            k += 1
        while j < k:
            rem = CHA - (pos % CHA)
            ncols_fit = (rem // g) & ~1
            if ncols_fit < 2:
                pos += rem  # pad to next sub boundary
                continue
            take = min(k - j, ncols_fit)
            runs.append((int(pos), int(j), int(take), int(g)))
            colpos[j:j + take] = pos + np.arange(take) * g
            pos += take * g
            j += take
    Epad = -(-pos // CHB) * CHB
    return runs, colpos, Epad


def _sched_from_runs(runs, Epad):
    """Group runs by chunk; a run never crosses a sub boundary."""
    nchunks = Epad // CHB
    by_chunk = [[] for _ in range(nchunks)]
    for (pos, jcol, n, g) in runs:
        chunk, rem = divmod(pos, CHB)
        sub, off = divmod(rem, CHA)
        assert off + n * g <= CHA
        by_chunk[chunk].append((sub, off, jcol, n, g))
    return by_chunk


def _balance_sides(src, dst, rounds=60, seed=0):
    """Side (+1=A, -1=B) per node so each dst's in-edges split ~evenly
    (gA = ceil(g/2) target) and |A| == N/2.  Greedy local search."""
    g = np.bincount(dst, minlength=N)
    target = (g & 1).astype(np.int64)
    k_out = np.bincount(src, minlength=N).astype(np.int64)
    o = np.argsort(g, kind="stable")
    s = np.empty(N, np.int64)
    s[o] = np.where(np.arange(N) % 2 == 0, 1, -1)
    rng = np.random.default_rng(seed)
    best_err, best_s = None, s.copy()
    for _ in range(rounds):
        dev = np.bincount(dst, weights=s[src].astype(np.float64),
                          minlength=N).astype(np.int64) - target
        err = int(np.abs(dev).sum())
        if best_err is None or err < best_err:
            best_err, best_s = err, s.copy()
        if err == 0:
            break
        W = np.bincount(src, weights=dev[dst].astype(np.float64),
                        minlength=N).astype(np.int64)
        gain = s * W - k_out
        candA = np.flatnonzero((s > 0) & (gain > 0))
        candB = np.flatnonzero((s < 0) & (gain > 0))
        m = min(len(candA), len(candB))
        if m == 0:
            break
        m = max(1, int(m * 0.35))
        s[rng.choice(candA, m, replace=False)] *= -1
        s[rng.choice(candB, m, replace=False)] *= -1
    return best_s


def preprocess(edge_index, x):
    src0 = np.asarray(edge_index[0], np.int64)
    dst0 = np.asarray(edge_index[1], np.int64)
    loop = np.arange(N, dtype=np.int64)
    src = np.concatenate([src0, loop])
    dst = np.concatenate([dst0, loop])

    deg = np.bincount(dst, minlength=N)
    dinv = (1.0 / np.sqrt(deg.astype(np.float64))).astype(np.float32)

    sides = _balance_sides(src, dst)
    a_mask = sides[src] > 0
    gA = np.bincount(dst[a_mask], minlength=N)
    gB = deg - gA
    gAp = gA + (gA & 1)   # even-padded phase degrees
    gBp = gB + (gB & 1)

    # core assignment: within each side, (gAp,gBp)-sorted round robin
    core_of = np.empty(N, np.int64)
    for sd, base in [(1, 0), (-1, 4)]:
        nodes = np.flatnonzero(sides == sd)
        o2 = nodes[np.lexsort((gBp[nodes], gAp[nodes]))]
        core_of[o2] = base + np.arange(len(o2)) % 4

    cnts = [int((core_of == c).sum()) for c in range(NCORES)]
    NC = -(-(2 + max(cnts)) // 512) * 512
    assert 4 * NC < IMAX

    # slots 0,1 reserved as zero rows; nodes at slots 2.. in lexsort order
    slot_node = []
    avec = np.zeros(NC, np.int64)
    bvec = np.zeros(NC, np.int64)
    for c in range(NCORES):
        nodes_c = np.flatnonzero(core_of == c)
        nodes_c = nodes_c[np.lexsort((gBp[nodes_c], gAp[nodes_c]))]
        sn = np.full(NC, -1, np.int64)
        sn[2:2 + len(nodes_c)] = nodes_c
        slot_node.append(sn)
        La = np.zeros(NC, np.int64)
        Lb = np.zeros(NC, np.int64)
        La[2:2 + len(nodes_c)] = gAp[nodes_c]
        Lb[2:2 + len(nodes_c)] = gBp[nodes_c]
        avec = np.maximum(avec, La)
        bvec = np.maximum(bvec, Lb)
    avec = np.maximum(avec, 2)   # no-memset: every slot reduces >= 2 entries
    bvec = np.maximum(bvec, 2)

    slot_of = np.full(N, -1, np.int64)
    for c in range(NCORES):
        real = slot_node[c] >= 0
        slot_of[slot_node[c][real]] = np.flatnonzero(real)

    TR = 2 + NCORES * NC
    baseB = max(0, TR - IMAX)
    row_of = 1 + core_of * NC + slot_of

    assert int(row_of[core_of < ACORES].max()) < IMAX
    assert int(row_of[core_of >= ACORES].min()) >= baseB

    zA = 1                      # core 0, slot 0 (always a zero row)
    zB = 1 + 7 * NC             # core 7, slot 0
    runsA, colposA, EA = _pack_phase(avec)
    runsB, colposB, EB = _pack_phase(bvec)
    chunksA = _sched_from_runs(runsA, EA)
    chunksB = _sched_from_runs(runsB, EB)

    idxA, idxB, dinvb_l, dinvt_l = [], [], [], []
    for c in range(NCORES):
        # --- A stream (default idx = zero row)
        sA = np.full(EA, zA, np.int64)
        m = (core_of[dst] == c) & a_mask
        es, cols = src[m], slot_of[dst[m]]
        o = np.argsort(cols, kind="stable")
        es, cols = es[o], cols[o]
        ranks = np.arange(len(cols)) - np.searchsorted(cols, cols)
        sA[colposA[cols] + ranks] = row_of[es]
        assert sA.max() < IMAX
        idxA.append(_wrap_idx(sA))

        # --- B stream (default = B zero row), direct slot columns
        sB = np.full(EB, zB - baseB, np.int64)
        m = (core_of[dst] == c) & (~a_mask)
        es, cols = src[m], slot_of[dst[m]]
        o = np.argsort(cols, kind="stable")
        es, cols = es[o], cols[o]
        ranks = np.arange(len(cols)) - np.searchsorted(cols, cols)
        sB[colposB[cols] + ranks] = row_of[es] - baseB
        assert sB.min() >= 0 and sB.max() < IMAX
        idxB.append(_wrap_idx(sB))

        dv = np.zeros(NC, np.float32)
        real = slot_node[c] >= 0
        dv[real] = dinv[slot_node[c][real]]
        dinvb_l.append(np.tile(dv[None, :].astype(F16), (128, 1)))
        dinvt_l.append(np.ascontiguousarray(
            dv.reshape(NC // 128, 128).T.astype(np.float32)))

    xt = np.zeros((TR, DIN), F16)
    xs = (np.asarray(x, np.float32) * dinv[:, None]).astype(F16)
    xt[row_of] = xs

    sched = {
        "NC": NC, "TR": TR, "baseB": baseB,
        "chunksA": chunksA, "chunksB": chunksB, "EA": EA, "EB": EB,
        "slot_node": slot_node,
    }
    data = {"xt": xt, "idxA": idxA, "idxB": idxB,
            "dinvb": dinvb_l, "dinvt": dinvt_l}
    return sched, data


# ----------------------------------------------------------------------------
# device kernel builder
# ----------------------------------------------------------------------------

def build_nc(sched, debug=False):
    NC, TR = sched["NC"], sched["TR"]
    baseB = sched["baseB"]
    EA, EB = sched["EA"], sched["EB"]
    fp32 = mybir.dt.float32
    f16 = mybir.dt.float16
    i16 = mybir.dt.int16
    AF = mybir.ActivationFunctionType
    OP = mybir.AluOpType
    NCH = NC // 512

    nc = bacc.Bacc("TRN2", target_bir_lowering=False, num_devices=NCORES,
                   num_swdge_queues=4, dynamic_dma_scratch_size=16384)

    xt_d = nc.dram_tensor("xt", [TR, DIN], f16, kind="ExternalInput")
    idxa_d = nc.dram_tensor("idxa", [128, EA // 16], i16, kind="ExternalInput")
    idxb_d = nc.dram_tensor("idxb", [128, EB // 16], i16, kind="ExternalInput")
    dinvb_d = nc.dram_tensor("dinvb", [128, NC], f16, kind="ExternalInput")
    w0_d = nc.dram_tensor("w0", [128, 256], f16, kind="ExternalInput")
    w1_d = nc.dram_tensor("w1", [128, 512], f16, kind="ExternalInput")
    w2_d = nc.dram_tensor("w2", [128, 256], f16, kind="ExternalInput")
    bnp_d = nc.dram_tensor("bnp", [128, 10], fp32, kind="ExternalInput")
    identb_d = nc.dram_tensor("identb", [128, 128], f16, kind="ExternalInput")
    identf_d = nc.dram_tensor("identf", [128, 128], fp32, kind="ExternalInput")
    out_d = nc.dram_tensor("out", [NC, DOUT], fp32, kind="ExternalOutput")
    dbg = {}
    if debug:
        for name, shape in [
            ("dbg_outA0", [128, NC]), ("dbg_outB0", [128, NC]),
            ("dbg_agg0", [128, NC]), ("dbg_h1", [128, 2 * NC]),
            ("dbg_agg1", [128, 2 * NC]), ("dbg_outA1", [128, 2 * NC]),
            ("dbg_outB1", [128, 2 * NC]),
        ]:
            dbg[name] = nc.dram_tensor(name, shape, fp32,
                                       kind="ExternalOutput")

    with tile.TileContext(nc) as tc:
        with (
            tc.tile_pool(name="const", bufs=1) as constp,
            tc.tile_pool(name="gat", bufs=3) as gatp,
            tc.tile_pool(name="gix", bufs=3) as gixp,
            tc.tile_pool(name="red", bufs=1) as redp,
            tc.tile_pool(name="agg", bufs=2) as aggp,
            tc.tile_pool(name="small", bufs=2) as smallp,
            tc.tile_pool(name="rowt", bufs=2) as rowp,
            tc.tile_pool(name="ps", bufs=3, space="PSUM") as psp,
            tc.tile_pool(name="pst", bufs=2, space="PSUM") as pstp,
            tc.tile_pool(name="pstf", bufs=2, space="PSUM") as pstfp,
            tc.tile_pool(name="dram", bufs=1, space="DRAM") as dramp,
        ):
            # ---- resident constants
            dinvb = constp.tile([128, NC], f16, tag="dinvb")
            w0 = constp.tile([128, 256], f16, tag="w0")
            w1 = constp.tile([128, 512], f16, tag="w1")
            w2 = constp.tile([128, 256], f16, tag="w2")
            bnp = constp.tile([128, 10], fp32, tag="bnp")
            identb = constp.tile([128, 128], f16, tag="identb")
            identf = constp.tile([128, 128], fp32, tag="identf")
            for t, d in [(dinvb, dinvb_d),
                         (w0, w0_d), (w1, w1_d),
                         (w2, w2_d), (bnp, bnp_d), (identb, identb_d),
                         (identf, identf_d)]:
                nc.sync.dma_start(t[:], d[:])

            tbl = dramp.tile([TR, DH], f16, tag="tbl", addr_space="Shared")
            tbl2 = dramp.tile([TR, DOUT], f16, tag="tbl2",
                              addr_space="Shared")
            agsrc = dramp.tile([NC, DH], f16, tag="agsrc")
            agsrc2 = dramp.tile([NC, DOUT], f16, tag="agsrc2")

            def gather_reduce(table, elem, blocks):
                """A+B gather phases -> (outA f16, outB f16) [128,2,*].

                elem: f16 elements per table row (128 or 256); a single
                gather call fetches the whole row (one descriptor/edge)."""
                outA = redp.tile([128, 2, NC], f16, tag="outA")
                outB = redp.tile([128, 2, NC], f16, tag="outB")
                with nc.allow_low_precision(reason="DVE accumulates fp32"):
                    chunksA, chunksB = sched["chunksA"], sched["chunksB"]
                    viewA = table[0:min(TR, IMAX), :]
                    viewB = table[baseB:TR, :]
                    work = []
                    for k in range(max(len(chunksA), len(chunksB))):
                        if k < len(chunksA):
                            work.append((idxa_d, viewA, outA, k, chunksA[k]))
                        if k < len(chunksB):
                            work.append((idxb_d, viewB, outB, k, chunksB[k]))
                    for (idxd, view, outX, k, chk) in work:
                        gbs = [gatp.tile([128, 2, CHA], f16,
                                         tag=f"gs{sx}", name=f"gs{sx}")
                               for sx in range(SUB)]
                        idxt = gixp.tile([128, CHB // 16], i16,
                                         tag="idxt")
                        nc.sync.dma_start(
                            idxt[:], idxd[:, k * (CHB // 16):
                                          (k + 1) * (CHB // 16)])
                        for sx in range(SUB):
                            nc.gpsimd.dma_gather(
                                out_ap=gbs[sx][:, :blocks, :],
                                in_ap=view,
                                idxs_ap=idxt[:, sx * (CHA // 16):
                                             (sx + 1) * (CHA // 16)],
                                num_idxs=CHA,
                                num_idxs_reg=CHA,
                                elem_size=elem,
                                transpose=True,
                                queue_num=sx % 4,
                            )
                        for (sub, off, ocol, n, g) in chk:
                            for j in range(blocks):
                                nc.vector.tensor_reduce(
                                    outX[:, j, ocol:ocol + n],
                                    gbs[sub][:, j, off:off + n * g]
                                    .rearrange("p (n g) -> p n g", g=g),
                                    axis=mybir.AxisListType.X,
                                    op=OP.add,
                                )
                return outA, outB

            def merge(outA, outB, blocks):
                """A+B add + dinv[dst] scale -> aggT f16 [128,2,NC]."""
                aggT = aggp.tile([128, 2, NC], f16, tag="aggbuf")
                for j in range(blocks):
                    nc.vector.tensor_tensor(aggT[:, j, :], outA[:, j, :],
                                            outB[:, j, :], OP.add)
                    nc.vector.tensor_tensor(aggT[:, j, :], aggT[:, j, :],
                                            dinvb[:], OP.mult)
                return aggT

            def bn_consts(st, blocks_out, bn_off, layer):
                """AllReduce stats -> per-feature scale A / bias B tiles."""
                stin = dramp.tile([128, 4], fp32, tag=f"stin{layer}")
                stout = dramp.tile([128, 4], fp32, tag=f"stout{layer}",
                                   addr_space="Shared")
                nc.gpsimd.dma_start(stin[:], st[:])
                nc.gpsimd.collective_compute(
                    "AllReduce", OP.add,
                    replica_groups=[list(range(NCORES))],
                    ins=[stin.opt()], outs=[stout.opt()],
                )
                stg = smallp.tile([128, 4], fp32, tag="stg")
                nc.sync.dma_start(stg[:], stout[:])
                b = blocks_out
                mu = smallp.tile([128, 2], fp32, tag="mu")
                va = smallp.tile([128, 2], fp32, tag="va")
                Ab = smallp.tile([128, 2], fp32, tag="Ab")
                Bb = smallp.tile([128, 2], fp32, tag="Bb")
                musq = smallp.tile([128, 2], fp32, tag="musq")
                rstd = smallp.tile([128, 2], fp32, tag="rstd")
                nc.vector.tensor_scalar(mu[:, :b], stg[:, 0:b], 1.0 / N, None,
                                        op0=OP.mult)
                nc.vector.tensor_scalar(va[:, :b], stg[:, 2:2 + b], 1.0 / N,
                                        None, op0=OP.mult)
                nc.vector.tensor_tensor(musq[:, :b], mu[:, :b], mu[:, :b],
                                        OP.mult)
                nc.vector.tensor_tensor(va[:, :b], va[:, :b], musq[:, :b],
                                        OP.subtract)
                sqv = smallp.tile([128, 2], fp32, tag="sqv")
                nc.vector.tensor_scalar(sqv[:, :b], va[:, :b], EPS, None,
                                        op0=OP.add)
                nc.scalar.activation(sqv[:, :b], sqv[:, :b], AF.Sqrt)
                nc.vector.reciprocal(rstd[:, :b], sqv[:, :b])
                gsl = bnp[:, bn_off:bn_off + b]
                bsl = bnp[:, bn_off + b:bn_off + 2 * b]
                nc.vector.tensor_tensor(Ab[:, :b], rstd[:, :b], gsl, OP.mult)
                nc.vector.tensor_tensor(Bb[:, :b], mu[:, :b], Ab[:, :b],
                                        OP.mult)
                nc.vector.tensor_tensor(Bb[:, :b], bsl, Bb[:, :b],
                                        OP.subtract)
                return Ab, Bb

            def conv_bn(aggT, wt, KT, bn_off, lrelu, layer):
                """matmul (out 2 blocks of 128) + BN(+lrelu) -> h f16."""
                cv = aggp.tile([128, 2, NC], f16, tag="aggbuf")
                ssum = smallp.tile([128, 2, NCH], fp32, tag="ssum")
                sqsum = smallp.tile([128, 2, NCH], fp32, tag="sqsum")
                for j in range(2):
                    for t in range(NCH):
                        ps = psp.tile([128, 512], fp32, tag="cps")
                        sl = slice(t * 512, (t + 1) * 512)
                        for kt in range(KT):
                            lhsT = wt[:, kt * 256 + j * 128:
                                      kt * 256 + (j + 1) * 128]
                            nc.tensor.matmul(ps[:], lhsT, aggT[:, kt, sl],
                                             start=(kt == 0),
                                             stop=(kt == KT - 1))
                        sq = smallp.tile([128, 512], f16, tag="sqd")
                        nc.scalar.activation(cv[:, j, sl], ps[:], AF.Copy,
                                             accum_out=ssum[:, j, t:t + 1])
                        nc.scalar.activation(sq[:], ps[:], AF.Square,
                                             accum_out=sqsum[:, j, t:t + 1])
                st = smallp.tile([128, 4], fp32, tag="stl")
                for j in range(2):
                    nc.vector.tensor_reduce(st[:, j:j + 1], ssum[:, j, :],
                                            axis=mybir.AxisListType.X,
                                            op=OP.add)
                    nc.vector.tensor_reduce(st[:, 2 + j:3 + j], sqsum[:, j, :],
                                            axis=mybir.AxisListType.X,
                                            op=OP.add)
                Ab, Bb = bn_consts(st, 2, bn_off, layer)
                h = aggp.tile([128, 2, NC], f16, tag="aggbuf")
                fn = AF.Lrelu if lrelu else AF.Identity
                for j in range(2):
                    nc.scalar.activation(h[:, j, :], cv[:, j, :], fn,
                                         bias=Bb[:, j:j + 1],
                                         scale=Ab[:, j:j + 1], alpha=SLOPE)
                return h, cv, st

            def write_rows(srcT, blocks, dst_dram, width, prescale):
                """(optional in-place dinv[src] prescale on DVE) + transpose
                + copy + DMA rows."""
                if prescale:
                    for j in range(blocks):
                        nc.vector.tensor_tensor(srcT[:, j, :], srcT[:, j, :],
                                                dinvb[:], OP.mult)
                for t in range(NC // 128):
                    row = rowp.tile([128, width], f16, tag="rowt")
                    for j in range(blocks):
                        pt = pstp.tile([128, 128], f16, tag="tps")
                        nc.tensor.transpose(
                            pt[:], srcT[:, j, t * 128:(t + 1) * 128],
                            identb[:])
                        nc.scalar.activation(row[:, j * 128:(j + 1) * 128],
                                             pt[:], AF.Copy)
                    nc.sync.dma_start(dst_dram[t * 128:(t + 1) * 128, :],
                                      row[:])

            # ================= layer 0 =================
            outA, outB = gather_reduce(xt_d, DIN, 1)
            if debug:
                nc.gpsimd.dma_start(dbg["dbg_outA0"][:], outA[:, 0, :])
                nc.gpsimd.dma_start(dbg["dbg_outB0"][:], outB[:, 0, :])
            aggT = merge(outA, outB, 1)
            if debug:
                nc.gpsimd.dma_start(dbg["dbg_agg0"][:], aggT[:, 0, :])
            h1, cv0, st0 = conv_bn(aggT, w0, 1, 0, True, 0)
            if debug:
                nc.gpsimd.dma_start(
                    dbg["dbg_h1"][:].rearrange("p (a b) -> p a b", a=2),
                    h1[:])
            write_rows(h1, 2, agsrc, DH, prescale=True)
            nc.gpsimd.collective_compute(
                "AllGather", OP.bypass,
                replica_groups=[list(range(NCORES))],
                ins=[agsrc.opt()], outs=[tbl[1:1 + NCORES * NC, :]],
            )

            # ================= layer 1 =================
            outA, outB = gather_reduce(tbl, DH, 2)
            if debug:
                nc.gpsimd.dma_start(
                    dbg["dbg_outA1"][:].rearrange("p (a b) -> p a b", a=2),
                    outA[:])
                nc.gpsimd.dma_start(
                    dbg["dbg_outB1"][:].rearrange("p (a b) -> p a b", a=2),
                    outB[:])
            aggT = merge(outA, outB, 2)
            if debug:
                nc.gpsimd.dma_start(
                    dbg["dbg_agg1"][:].rearrange("p (a b) -> p a b", a=2),
                    aggT[:])
            h2, _, _ = conv_bn(aggT, w1, 2, 4, True, 1)
            # transform-first for layer 2: T2 = W2 @ (dinv * h2)
            hs2 = aggp.tile([128, 2, NC], f16, tag="aggbuf")
            for j in range(2):
                nc.vector.tensor_tensor(hs2[:, j, :], h2[:, j, :], dinvb[:],
                                        OP.mult)
            t2 = aggp.tile([128, 2, NC], f16, tag="aggbuf")
            for t in range(NCH):
                ps = psp.tile([128, 512], fp32, tag="cps")
                sl = slice(t * 512, (t + 1) * 512)
                for kt in range(2):
                    nc.tensor.matmul(ps[:], w2[:, kt * 128:(kt + 1) * 128],
                                     hs2[:, kt, sl],
                                     start=(kt == 0), stop=(kt == 1))
                nc.scalar.activation(t2[:, 0, sl], ps[:], AF.Copy)
            write_rows(t2, 1, agsrc2, DOUT, prescale=False)
            nc.gpsimd.collective_compute(
                "AllGather", OP.bypass,
                replica_groups=[list(range(NCORES))],
                ins=[agsrc2.opt()], outs=[tbl2[1:1 + NCORES * NC, :]],
            )

            # ================= layer 2 =================
            outA, outB = gather_reduce(tbl2, DOUT, 1)
            aggT = merge(outA, outB, 1)
            # aggT IS the conv output (transform-first); BN only, no lrelu.
            ssum = smallp.tile([128, 2, NCH], fp32, tag="ssum")
            sqsum = smallp.tile([128, 2, NCH], fp32, tag="sqsum")
            for t in range(NCH):
                sl = slice(t * 512, (t + 1) * 512)
                sq = smallp.tile([128, 512], f16, tag="sqd")
                nc.scalar.activation(sq[:], aggT[:, 0, sl], AF.Square,
                                     accum_out=sqsum[:, 0, t:t + 1])
                nc.vector.tensor_reduce(ssum[:, 0, t:t + 1], aggT[:, 0, sl],
                                        axis=mybir.AxisListType.X, op=OP.add)
            st = smallp.tile([128, 4], fp32, tag="stl")
            nc.vector.tensor_reduce(st[:, 0:1], ssum[:, 0, :],
                                    axis=mybir.AxisListType.X, op=OP.add)
            nc.vector.tensor_reduce(st[:, 2:3], sqsum[:, 0, :],
                                    axis=mybir.AxisListType.X, op=OP.add)
            nc.vector.memset(st[:, 1:2], 0)
            nc.vector.memset(st[:, 3:4], 0)
            Ab, Bb = bn_consts(st, 1, 8, 2)
            for t in range(NC // 128):
                hf = smallp.tile([128, 128], fp32, tag="hfin")
                nc.scalar.activation(hf[:], aggT[:, 0, t * 128:(t + 1) * 128],
                                     AF.Identity,
                                     bias=Bb[:, 0:1], scale=Ab[:, 0:1])
                row = rowp.tile([128, DOUT], fp32, tag="rowtf")
                pt = pstfp.tile([128, 128], fp32, tag="tpsf")
                nc.tensor.transpose(pt[:], hf[:], identf[:])
                nc.vector.tensor_copy(row[:], pt[:])
                nc.sync.dma_start(out_d[t * 128:(t + 1) * 128, :], row[:])

    nc.compile()
    return nc


# ----------------------------------------------------------------------------
# entry point
# ----------------------------------------------------------------------------

def _make_inmaps(sched, data, W0, W1, W2, g0, be0, g1, be1, g2, be2):
    w0 = np.ascontiguousarray(W0.T.astype(F16))
    w1 = np.ascontiguousarray(
        W1.T.reshape(2, 128, 256).transpose(1, 0, 2).reshape(128, 512)
        .astype(F16))
    w2 = np.ascontiguousarray(
        W2.T.reshape(2, 128, 128).transpose(1, 0, 2).reshape(128, 256)
        .astype(F16))
    bnp = np.zeros((128, 10), np.float32)
    bnp[:, 0:2] = g0.reshape(2, 128).T
    bnp[:, 2:4] = be0.reshape(2, 128).T
    bnp[:, 4:6] = g1.reshape(2, 128).T
    bnp[:, 6:8] = be1.reshape(2, 128).T
    bnp[:, 8] = g2
    bnp[:, 9] = be2
    identb = np.eye(128, dtype=F16)
    identf = np.eye(128, dtype=np.float32)
    maps = []
    for c in range(NCORES):
        maps.append({
            "xt": data["xt"], "idxa": data["idxA"][c],
            "idxb": data["idxB"][c],
            "dinvb": data["dinvb"][c],
            "w0": w0, "w1": w1, "w2": w2, "bnp": bnp,
            "identb": identb, "identf": identf,
        })
    return maps


_CACHE = {}


def kernel(x, edge_index, W0, b0, g0, be0, W1, b1, g1, be1, W2, b2, g2, be2,
           _trace=False, _tmpdir=None, _debug=False):
    x = np.asarray(x, np.float32)
    edge_index = np.asarray(edge_index, np.int32)
    args = [np.asarray(a, np.float32)
            for a in (W0, b0, g0, be0, W1, b1, g1, be1, W2, b2, g2, be2)]
    (W0, b0, g0, be0, W1, b1, g1, be1, W2, b2, g2, be2) = args
    # conv bias cancels exactly in training-mode BatchNorm -> ignored.

    key = (edge_index.tobytes()[:256], int(edge_index.sum()), bool(_debug))
    if key not in _CACHE:
        sched, data = preprocess(edge_index, x)
        nc_obj = build_nc(sched, debug=_debug)
        _CACHE[key] = (sched, nc_obj)
    else:
        sched, nc_obj = _CACHE[key]
        _, data = preprocess(edge_index, x)

    in_maps = _make_inmaps(sched, data, W0, W1, W2, g0, be0, g1, be1, g2, be2)
    res = run_bass_kernel_spmd(nc_obj, in_maps, core_ids=list(range(NCORES)),
                               trace=_trace, tmpdir=_tmpdir)

    out = np.zeros((N, DOUT), np.float32)
    for c in range(NCORES):
        o = np.asarray(res.results[c]["out"])
        sn = sched["slot_node"][c]
        real = sn >= 0
        out[sn[real]] = o[real]
    kernel._last_result = res
    kernel._last_sched = sched
    return out
